# revision 36
# baseline (speedup 1.0000x reference)
"""CapsuleLayer dynamic-routing kernel for 8 Trainium2 NeuronCores.

Problem: inputs [64, 4096, 8] f32, W [32, 4096, 16, 8] f32.
  hat[b,c,n,j] = sum_i W[c,n,j,i] * x[b,n,i]
  3 routing iterations: c = softmax_C(b); out = squash(sum_n c*hat);
  b += <out, hat>.

Strategy: shard the n (input-capsule) axis across the 8 cores
(N_loc = 512/core); everything stays SBUF-resident; hat is never
materialized.  Per routing iteration t>0:
  - logits: A[(c2,b),(n,i)] = sum_j out*W via block-diagonal matmuls
    (BD lhsT built from transposed osum each iteration), then an
    elementwise multiply by x (split across DVE-direct-from-PSUM /
    ACT-drain+DVE-bf16 / ACT-drain+Pool paths, tunable), and the
    i-reduction is folded into PE transpose-accumulation: 8 strided
    [128,128] transposes of the product accumulate in PSUM, yielding
    beta^T [128n, (c2,b)] directly (f32 accumulation, no add tree).
  - exp straight from PSUM into the e-table (ACT); softmax Z is a
    bf16 pairwise add tree on DVE whose first level is emitted
    per-unit-pair during the beta phase; 1/Z is folded into x.
  - s-matmul is FLIPPED: out[64b, 16j] tiles = Rg[128n, 64b]^T @
    W3[128n, 16j], accumulating 32 (i, nt) steps per (cb, c8) into a
    PSUM region that is ALREADY the final [B, (c, j)] layout -- no
    transposes or per-capsule drains; one ACT copy per cb.
  - one [64, 512] f32 AllReduce per iteration, then a DVE-only squash
    (bit-trick rsqrt + 2 Newton steps; no ACT table switching -- the
    only ACT table used is exp_and_others).
t=0 (uniform couplings) is the same flipped matmul with x as lhsT and
W3 [128, (c8, j)] as rhs.  Logits are recomputed each iteration from
the running sum of outputs (the update is linear), so no [B,C,N] state.

Scheduling: the iteration boundary is pipelined per capsule-block --
s-matmul(t, cb) -> drain -> AllReduce(cb) -> squash(cb) -> osum(cb) ->
BD(g=cb) -> iteration t+1's beta units for group g=cb, so the next
iteration's beta phase overlaps the current s-phase.  The only global
barrier per iteration is the softmax normalizer Z (it needs all 32
capsules' exp tables).  PSUM budget is exactly 8 banks: 2x pA
[128,1024] + 2x beta^T [128,NT,128] + osumT + s-accumulator.
"""

import contextlib

import numpy as np

_nullctx = contextlib.nullcontext

B, N, I = 64, 4096, 8
C, D = 32, 16
ROUTINGS = 3
EPS = 1e-7
NCORES = 8
NL = N // NCORES          # 512 n per core
NT = NL // 128            # 4 partition tiles of n
CHUNKS = NL * I // 512    # 8 chunks of 512 along flat (n,i)


# ---------------------------------------------------------------------------
# Host-side layout prep (pure numpy, per core)
# ---------------------------------------------------------------------------

_CONSTS = {}


def _prep_consts():
    if not _CONSTS:
        import ml_dtypes
        _CONSTS["eyef"] = np.eye(128, dtype=np.float32)
        _CONSTS["eyeb"] = np.eye(128, dtype=np.float32).astype(ml_dtypes.bfloat16)
        _CONSTS["bdmask"] = _bd_mask().astype(ml_dtypes.bfloat16)
    return _CONSTS


def host_prep(x, W, k):
    """Per-core input layouts for core k (n slice [k*NL, (k+1)*NL))."""
    n0 = k * NL
    Wk = np.ascontiguousarray(W[:, n0:n0 + NL])          # [C, NL, D, I]
    xk = np.ascontiguousarray(x[:, n0:n0 + NL])          # [B, NL, I]

    # W2 [128=(cp*16+j), (cg, n*8+i)]  = W[cg*8+cp, n, j, i]   (bf16)
    w2 = Wk.reshape(4, 8, NL, D, I).transpose(1, 3, 0, 2, 4).reshape(128, 4 * NL * I)
    # W3 [128=nn, (cb, nt, i, c8, j)] = W[cb*8+c8, nt*128+nn, j, i]  (bf16)
    w3 = Wk.reshape(4, 8, NT, 128, D, I).transpose(3, 0, 2, 5, 1, 4) \
           .reshape(128, NT * I * C * D)
    # xt3 [128=nn, (i, nt, b)] = x[b, nt*128+nn, i]             (bf16)
    xt3 = xk.reshape(B, NT, 128, I).transpose(2, 3, 1, 0).reshape(128, I * NT * B)
    # xr2 [64=b, (n*8+i)] = x[b, n, i]  (bf16; device duplicates rows)
    xr2 = xk.reshape(B, NL * I)

    import ml_dtypes
    bf = ml_dtypes.bfloat16
    cst = _prep_consts()
    return {
        "w2": w2.astype(bf),
        "w3": w3.astype(bf),
        "xt3": xt3.astype(bf),
        "xr2": xr2.astype(bf),
        "eyef": cst["eyef"],
        "eyeb": cst["eyeb"],
        "bdmask": cst["bdmask"],
    }


def host_prep_all(x, W):
    """Vectorized host_prep for all cores at once."""
    import ml_dtypes
    bf = ml_dtypes.bfloat16
    Wb = np.ascontiguousarray(W, dtype=np.float32).astype(bf)   # [C, N, D, I]
    xb = np.ascontiguousarray(x, dtype=np.float32).astype(bf)   # [B, N, I]
    K = NCORES
    w2 = Wb.reshape(4, 8, K, NL, D, I).transpose(2, 1, 4, 0, 3, 5) \
           .reshape(K, 128, 4 * NL * I)
    w3 = Wb.reshape(4, 8, K, NT, 128, D, I).transpose(2, 4, 0, 3, 6, 1, 5) \
           .reshape(K, 128, NT * I * C * D)
    xt3 = xb.reshape(B, K, NT, 128, I).transpose(1, 3, 4, 2, 0) \
            .reshape(K, 128, I * NT * B)
    xr2 = xb.reshape(B, K, NL * I).transpose(1, 0, 2)           # [k, 64, 4096]
    cst = _prep_consts()
    return [
        {"w2": np.ascontiguousarray(w2[k]),
         "w3": np.ascontiguousarray(w3[k]),
         "xt3": np.ascontiguousarray(xt3[k]),
         "xr2": np.ascontiguousarray(xr2[k]),
         "eyef": cst["eyef"], "eyeb": cst["eyeb"], "bdmask": cst["bdmask"]}
        for k in range(K)
    ]


def _bd_mask():
    # mask[r, col] = 1 where ((r%32)//16) == col//64 — selects which b-half
    # of a block-diagonal lhsT tile each 16-row (one capsule's j-block) feeds.
    r = np.arange(128)[:, None]
    col = np.arange(128)[None, :]
    return (((r % 32) // 16) == (col // 64)).astype(np.float32)


# ---------------------------------------------------------------------------
# Bass device program
# ---------------------------------------------------------------------------

_CACHE = {}

# x-mul path per quarter slot (64 quarters/iter = 16 units x 4):
#   D = DVE direct from PSUM (f32 in, bf16 out, 1x)
#   V = ACT drain to bf16 + DVE mul (2x)
#   P = ACT drain to bf16 + Pool mul
# per-iteration pattern: the t=1 beta phase overlaps t0 (ACT busy with
# t0/xfer work -> D-heavier); the t=2 beta phase overlaps the t=1
# s-phase (DVE busy with Rg -> V-heavy)
QPATH_BY_T = {
    1: "D P V V P D V P V P D V P V P D".split(),
    2: "D P V V P D V P V P D V P V P D".split(),
}
# (cb, i) Rg-mul subunits assigned to Pool instead of DVE, per t
RG_POOL_T = {
    1: {(0, 3), (1, 3), (2, 3), (3, 3), (0, 6)},
    2: {(0, 3), (1, 3), (2, 3), (3, 3), (0, 6), (1, 6), (2, 6), (3, 6)},
}

MAGIC = 0x5F3759DF

# targeted scheduler-priority boosts (tested individually; blanket
# boosting everything regressed)
HP_FLUSHZ = False
HP_TAIL = False
HP_EXP = False
# emit Pool Rg-muls as two nt-half instructions (finer interleave)
POOL_RG_SPLIT = True
WP_BUFS = 2
RP_BUFS = 2


def _build_nc(sim=False):
    import concourse.bass as bass
    import concourse.bacc as bacc
    import concourse.mybir as mybir
    import concourse.tile as tile

    dt = mybir.dt
    f32, bf16, i32 = dt.float32, dt.bfloat16, dt.int32
    ALU = mybir.AluOpType
    AF = mybir.ActivationFunctionType
    AX = mybir.AxisListType

    nc = bacc.Bacc("TRN2", target_bir_lowering=False, debug=False,
                   num_devices=NCORES)

    w2_d = nc.dram_tensor("w2", [128, 4 * NL * I], bf16, kind="ExternalInput").ap()
    w3_d = nc.dram_tensor("w3", [128, NT * I * C * D], bf16, kind="ExternalInput").ap()
    xt3_d = nc.dram_tensor("xt3", [128, I * NT * B], bf16, kind="ExternalInput").ap()
    xr2_d = nc.dram_tensor("xr2", [B, NL * I], bf16, kind="ExternalInput").ap()
    eyef_d = nc.dram_tensor("eyef", [128, 128], f32, kind="ExternalInput").ap()
    eyeb_d = nc.dram_tensor("eyeb", [128, 128], bf16, kind="ExternalInput").ap()
    bdm_d = nc.dram_tensor("bdmask", [128, 128], bf16, kind="ExternalInput").ap()
    out_d = nc.dram_tensor("out", [B, C * D], f32, kind="ExternalOutput").ap()

    with tile.TileContext(nc) as tc:
        with (
            tc.tile_pool(name="const", bufs=1) as cp,
            tc.tile_pool(name="work", bufs=WP_BUFS) as wp,
            tc.tile_pool(name="rg", bufs=RP_BUFS) as rp,
            tc.tile_pool(name="dram", bufs=2, space="DRAM") as dp,
        ):
            sW2 = cp.tile([128, 4, CHUNKS, 512], bf16)
            sW3 = cp.tile([128, 4, NT, I, 8, D], bf16)
            sXT3 = cp.tile([128, I, NT, B], bf16)
            sXR2 = cp.tile([128, CHUNKS, 512], bf16)
            sEyeF = cp.tile([128, 128], f32)
            sEyeB = cp.tile([128, 128], bf16)
            sBdm = cp.tile([128, 128], bf16)

            # DMA-in: single queue in strict priority order — DMA transfers
            # serialize on the shared DMA-engine device, so arrival order is
            # consumption order: t0-cb0 inputs, then xr2/w2g0 (needed by the
            # first beta block), then alternating w3-cb / w2-g.
            w3v = sW3[:].rearrange("p cb a b c d -> p cb (a b c d)")
            qsz = NT * I * 8 * D
            w2v = sW2[:].rearrange("p g a b -> p g (a b)")
            gsz = CHUNKS * 512
            xrv = sXR2[:].rearrange("p a b -> p (a b)")
            nc.sync.dma_start(sXT3[:].rearrange("p a b c -> p (a b c)"), xt3_d[:])
            nc.sync.dma_start(sEyeB[:], eyeb_d[:])
            nc.sync.dma_start(sEyeF[:], eyef_d[:])
            nc.sync.dma_start(sBdm[:], bdm_d[:])
            ssz = qsz // 4
            for s_ in range(4):
                nc.sync.dma_start(w3v[:, 0, s_ * ssz:(s_ + 1) * ssz],
                                  w3_d[:, s_ * ssz:(s_ + 1) * ssz])
            nc.sync.dma_start(xrv[0:B, :], xr2_d[:])
            nc.sync.dma_start(xrv[B:128, :], xr2_d[:])
            nc.sync.dma_start(w2v[:, 0, :], w2_d[:, 0:gsz])
            for q_ in range(1, 4):
                nc.sync.dma_start(w3v[:, q_, :],
                                  w3_d[:, q_ * qsz:(q_ + 1) * qsz])
                nc.sync.dma_start(w2v[:, q_, :],
                                  w2_d[:, q_ * gsz:(q_ + 1) * gsz])

            sET = cp.tile([128, NT, C, B], bf16)
            sXt = cp.tile([128, I, NT, B], bf16)
            zA = cp.tile([128, NT, 16, B], bf16)
            zB = cp.tile([128, NT, 8, B], bf16)
            sZ = cp.tile([128, NT, B], bf16)
            sZr = cp.tile([128, NT, B], bf16)
            sS = cp.tile([B, C * D], f32)
            sSr = cp.tile([B, C * D], f32)
            sOut = cp.tile([B, C * D], f32)
            sOsum = cp.tile([B, C * D], f32)
            sOsumT = cp.tile([128, 4, B], bf16)
            sBDall = cp.tile([128, 16, 128], bf16)
            # squash temps (DVE-only; rsqrt via bit trick + Newton)
            sq = cp.tile([B, C * D], f32)
            s2 = cp.tile([B, C], f32)
            s2e = cp.tile([B, C], f32)
            ry = cp.tile([B, C], f32)
            rt = cp.tile([B, C], f32)
            opp = cp.tile([B, C], f32)
            rden = cp.tile([B, C], f32)
            fac = cp.tile([B, C], f32)

            nc.vector.memset(sBDall[:], 0.0)

            def squash_cb(src, dst, cb):
                sl = slice(cb * 128, (cb + 1) * 128)
                cs = slice(cb * 8, (cb + 1) * 8)
                nc.vector.tensor_mul(sq[:, sl], src[:, sl], src[:, sl])
                nc.vector.tensor_reduce(
                    s2[:, cs], sq[:, sl].rearrange("b (c j) -> b c j", j=D),
                    axis=AX.X, op=ALU.add)
                nc.vector.tensor_scalar_add(s2e[:, cs], s2[:, cs], EPS)
                # rsqrt(s2e): quake seed + 2 Newton steps (DVE-only, so ACT
                # never switches activation tables away from exp)
                yi = ry[:, cs].bitcast(i32)
                xi = s2e[:, cs].bitcast(i32)
                nc.vector.tensor_scalar(yi, xi, 1, None,
                                        op0=ALU.logical_shift_right)
                nc.vector.tensor_scalar(yi, yi, -1, MAGIC,
                                        op0=ALU.mult, op1=ALU.add)
                for _ in range(2):
                    nc.vector.tensor_mul(rt[:, cs], ry[:, cs], ry[:, cs])
                    nc.vector.tensor_mul(rt[:, cs], rt[:, cs], s2e[:, cs])
                    nc.vector.tensor_scalar(rt[:, cs], rt[:, cs], -0.5, 1.5,
                                            op0=ALU.mult, op1=ALU.add)
                    nc.vector.tensor_mul(ry[:, cs], ry[:, cs], rt[:, cs])
                nc.vector.tensor_scalar_add(opp[:, cs], s2[:, cs], 1.0)
                nc.vector.reciprocal(rden[:, cs], opp[:, cs])
                nc.vector.tensor_mul(fac[:, cs], s2[:, cs], ry[:, cs])
                nc.vector.tensor_mul(fac[:, cs], fac[:, cs], rden[:, cs])
                fb = fac[:, cs].rearrange("b (c o) -> b c o", o=1) \
                    .broadcast_to([B, 8, D])
                nc.vector.tensor_mul(
                    dst[:, sl].rearrange("b (c j) -> b c j", j=D),
                    src[:, sl].rearrange("b (c j) -> b c j", j=D), fb)

            def all_reduce_cb(cb):
                sl = slice(cb * 128, (cb + 1) * 128)
                if sim:
                    nc.vector.tensor_copy(sSr[:, sl], sS[:, sl])
                else:
                    di = dp.tile([B, 128], f32, tag="ar_in")
                    do = dp.tile([B, 128], f32, tag="ar_out")
                    nc.sync.dma_start(di[:], sS[:, sl])
                    nc.gpsimd.collective_compute(
                        "AllReduce", mybir.AluOpType.add,
                        replica_groups=[list(range(NCORES))],
                        ins=[di[:].opt()], outs=[do[:].opt()])
                    nc.sync.dma_start(sSr[:, sl], do[:])

            # --- pipelined schedule -----------------------------------
            # per capsule-block cb: s-matmul(t, cb) -> drain -> AllReduce
            # -> squash -> osum -> BD(g=cb) -> NEXT iteration's beta units
            # for g=cb.  The only global barrier per iteration is softmax Z.

            with (
                tc.tile_pool(name="psA", bufs=2, space="PSUM") as psA,
                tc.tile_pool(name="psT", bufs=2, space="PSUM") as psT,
                tc.tile_pool(name="psS", bufs=1, space="PSUM") as psS,
            ):
                pending = [None]

                def emit_reduce():
                    if pending[0] is None:
                        return
                    g, p, pT2, tmp = pending[0]
                    pending[0] = None
                    c0 = g * 8 + 2 * p
                    with (tc.high_priority() if HP_EXP
                          else _nullctx()):
                        nc.scalar.activation(
                            sET[:, :, c0:c0 + 2, :]
                               .rearrange("p nt a b -> p nt (a b)"),
                            pT2[:].rearrange("p nt f -> p (nt f)"), AF.Exp)
                        if g >= 2:
                            # level-1 Z: e[c] + e[c+16] for this p's pair
                            cl = (g - 2) * 8 + 2 * p
                            nc.vector.tensor_add(
                                zA[:, :, cl:cl + 2, :],
                                sET[:, :, cl:cl + 2, :],
                                sET[:, :, cl + 16:cl + 18, :])

                def emit_traccum_q(q):
                    g, p, pT2, tmp = pending[0]
                    t8 = tmp[:, 2 * q:2 * q + 2, :] \
                        .rearrange("p a b -> p (a b)") \
                        .rearrange("p (n i) -> p n i", i=I)
                    for i in range(I):
                        nc.tensor.matmul(
                            pT2[:, q, :], t8[:, :, i], sEyeB[:],
                            start=(i == 0), stop=(i == I - 1))

                def emit_pb_unit(g, p, qpath):
                    # one beta unit (capsule pair) of group g
                    u = g * 4 + p
                    pT2 = psT.tile([128, NT, 128], f32, name="pT2",
                                   tag="bT")
                    tmp = wp.tile([128, CHUNKS, 512], bf16, name="tmp",
                                  tag="tmp")
                    for q in range(4):
                        pA = psA.tile([128, 1024], f32, name="pA",
                                      tag="pA")
                        for h in range(2):
                            nc.tensor.matmul(
                                pA[:, 512 * h:512 * (h + 1)],
                                sBDall[:, u, :],
                                sW2[:, g, 2 * q + h, :],
                                start=True, stop=True)
                        path = qpath[(u * 4 + q) % 16]
                        tv = tmp[:, 2 * q:2 * q + 2, :] \
                            .rearrange("p a b -> p (a b)")
                        xv = sXR2[:, 2 * q:2 * q + 2, :] \
                            .rearrange("p a b -> p (a b)")
                        if path == "D":
                            nc.vector.tensor_mul(tv, pA[:], xv)
                        else:
                            nc.scalar.copy(tv, pA[:])
                            meng = nc.gpsimd if path == "P" else nc.vector
                            meng.tensor_mul(tv, tv, xv)
                        # previous unit's quarter-q i-reduce fills PE
                        # while this unit's x-mul is still running
                        if pending[0] is not None:
                            emit_traccum_q(q)
                            if q == 3:
                                emit_reduce()
                    pending[0] = (g, p, pT2, tmp)

                def flush_z():
                    if pending[0] is not None:
                        for q in range(4):
                            emit_traccum_q(q)
                        emit_reduce()
                    hpz = tc.high_priority() if HP_FLUSHZ else _nullctx()
                    hpz.__enter__()
                    nc.vector.tensor_add(zB[:], zA[:, :, 0:8, :],
                                         zA[:, :, 8:16, :])
                    nc.vector.tensor_add(zA[:, :, 0:4, :], zB[:, :, 0:4, :],
                                         zB[:, :, 4:8, :])
                    nc.vector.tensor_add(zB[:, :, 0:2, :], zA[:, :, 0:2, :],
                                         zA[:, :, 2:4, :])
                    nc.vector.tensor_add(
                        sZ[:].rearrange("p nt (o b) -> p nt o b", o=1),
                        zB[:, :, 0:1, :], zB[:, :, 1:2, :])
                    with nc.allow_low_precision(reason="Z~32 in bf16"):
                        nc.vector.reciprocal(sZr[:], sZ[:])
                    for h in range(2):
                        nt0, nt1 = h * 2, h * 2 + 2
                        zb = sZr[:, nt0:nt1, :] \
                            .rearrange("p (o nt) b -> p o nt b", o=1) \
                            .broadcast_to([128, I, 2, B])
                        nc.vector.tensor_mul(sXt[:, :, nt0:nt1, :],
                                             sXT3[:, :, nt0:nt1, :], zb)
                    hpz.__exit__(None, None, None)

                slot_rgs = {}

                def emit_rg(t, cb, irange):
                    # coupling * x products feeding the cb s-matmul
                    rgs = slot_rgs.setdefault(cb, [None] * I)
                    for i in irange:
                        rg = rp.tile([128, NT, 8, B], bf16,
                                     name=f"rg{i}", tag=f"rg{i}")
                        xb = sXt[:, i, :, :] \
                            .rearrange("p nt (o b) -> p nt o b", o=1) \
                            .broadcast_to([128, NT, 8, B])
                        pool = (cb, i) in RG_POOL_T[t]
                        if pool and POOL_RG_SPLIT:
                            for h_ in range(2):
                                ns = slice(2 * h_, 2 * h_ + 2)
                                nc.gpsimd.tensor_mul(
                                    rg[:, ns, :, :],
                                    sET[:, ns, cb * 8:(cb + 1) * 8, :],
                                    xb[:, ns, :, :])
                        else:
                            meng = nc.gpsimd if pool else nc.vector
                            meng.tensor_mul(
                                rg[:], sET[:, :, cb * 8:(cb + 1) * 8, :], xb)
                        rgs[i] = rg

                def emit_smm(t, cb):
                    # s-matmul for capsule block cb at routing step t
                    pS = psS.tile([B, 128], f32, name="pS", tag="sS")
                    if t == 0:
                        # nt-outer so cb0's first w3 sub-chunk DMA unblocks
                        # the first steps
                        step = 0
                        for nt in range(NT):
                            for i in range(I):
                                rhs = sW3[:, cb, nt, i, :, :] \
                                    .rearrange("p a b -> p (a b)")
                                nc.tensor.matmul(
                                    pS[:], sXT3[:, i, nt, :], rhs,
                                    start=(step == 0), stop=(step == 31))
                                step += 1
                        nc.scalar.mul(sS[:, cb * 128:(cb + 1) * 128], pS[:],
                                      1.0 / C)
                        return
                    rgs = slot_rgs.pop(cb)
                    # one accumulation group open per PSUM tile at a time
                    for c8 in range(8):
                        for i in range(I):
                            for nt in range(NT):
                                nc.tensor.matmul(
                                    pS[:, c8 * D:(c8 + 1) * D],
                                    rgs[i][:, nt, c8, :],
                                    sW3[:, cb, nt, i, c8, :],
                                    start=(i == 0 and nt == 0),
                                    stop=(i == I - 1 and nt == NT - 1))
                    nc.scalar.copy(sS[:, cb * 128:(cb + 1) * 128], pS[:])

                def emit_pd(t, cb):
                    if t > 0:
                        emit_rg(t, cb, range(I))
                    emit_smm(t, cb)

                def emit_tail(t, cb):
                    last = (t == ROUTINGS - 1)
                    hpt = tc.high_priority() if HP_TAIL else _nullctx()
                    hpt.__enter__()
                    all_reduce_cb(cb)
                    sl = slice(cb * 128, (cb + 1) * 128)
                    if t == 0:
                        squash_cb(sSr, sOsum, cb)
                    else:
                        squash_cb(sSr, sOut, cb)
                        if not last:
                            nc.vector.tensor_add(sOsum[:, sl], sOsum[:, sl],
                                                 sOut[:, sl])
                    if last:
                        nc.sync.dma_start(out_d[:, sl], sOut[:, sl])
                        hpt.__exit__(None, None, None)
                        return
                    # transpose osum block, build BD tiles for group g=cb
                    pT = psT.tile([128, 4, B], f32, name="pT", tag="ot",
                                  bufs=1)
                    nc.tensor.transpose(pT[:, cb, :], sOsum[:, sl],
                                        sEyeF[0:B, 0:B])
                    nc.scalar.copy(sOsumT[:, cb, :], pT[:, cb, :])
                    g = cb
                    for p in range(4):
                        ob = sOsumT[32 * p:32 * p + 32, g, :] \
                            .rearrange("p (o b) -> p o b", o=1) \
                            .broadcast_to([32, 2, B])
                        nc.vector.tensor_mul(
                            sBDall[32 * p:32 * p + 32, g * 4 + p, :]
                                .rearrange("p (h b) -> p h b", h=2),
                            ob,
                            sBdm[32 * p:32 * p + 32, :]
                                .rearrange("p (h b) -> p h b", h=2))
                    hpt.__exit__(None, None, None)

                for t in range(ROUTINGS):
                    emit_pd(t, 0)
                    for cb in range(4):
                        emit_tail(t, cb)
                        if t < ROUTINGS - 1:
                            # next iteration's beta units for g=cb, with the
                            # next cb's Rg/s-matmul interleaved so no engine
                            # queue sees head-of-line blocking
                            nxt = cb + 1 if cb < 3 else None
                            for p in range(4):
                                emit_pb_unit(g=cb, p=p,
                                             qpath=QPATH_BY_T[t + 1])
                            if nxt is not None:
                                if t > 0:
                                    emit_rg(t, nxt, range(I))
                                emit_smm(t, nxt)
                        elif cb < 3:
                            emit_pd(t, cb + 1)
                    if t < ROUTINGS - 1:
                        flush_z()
    nc.compile()
    return nc


def get_nc(sim=False):
    key = "nc_sim" if sim else "nc"
    if key not in _CACHE:
        _CACHE[key] = _build_nc(sim=sim)
    return _CACHE[key]


def kernel(inputs, W):
    inputs = np.asarray(inputs, dtype=np.float32)
    W = np.asarray(W, dtype=np.float32)
    nc = get_nc()
    in_maps = host_prep_all(inputs, W)
    from concourse import bass_utils
    res = bass_utils.run_bass_kernel_spmd(
        nc, in_maps, core_ids=list(range(NCORES)))
    return res.results[0]["out"].reshape(B, C, D).astype(np.float32)


# revision 38
# speedup vs baseline: 1.0747x; 1.0747x over previous
"""CapsuleLayer dynamic-routing kernel for 8 Trainium2 NeuronCores.

Problem: inputs [64, 4096, 8] f32, W [32, 4096, 16, 8] f32.
  hat[b,c,n,j] = sum_i W[c,n,j,i] * x[b,n,i]
  3 routing iterations: c = softmax_C(b); out = squash(sum_n c*hat);
  b += <out, hat>.

Strategy: shard the n (input-capsule) axis across the 8 cores
(N_loc = 512/core); everything stays SBUF-resident; hat is never
materialized.  Per routing iteration t>0:
  - logits: A[(c2,b),(n,i)] = sum_j out*W via block-diagonal matmuls
    (BD lhsT built from transposed osum each iteration), then an
    elementwise multiply by x (split across DVE-direct-from-PSUM /
    ACT-drain+DVE-bf16 / ACT-drain+Pool paths, tunable), and the
    i-reduction is folded into PE transpose-accumulation: 8 strided
    [128,128] transposes of the product accumulate in PSUM, yielding
    beta^T [128n, (c2,b)] directly (f32 accumulation, no add tree).
  - exp straight from PSUM into the e-table (ACT); softmax Z is a
    bf16 pairwise add tree on DVE whose first level is emitted
    per-unit-pair during the beta phase; 1/Z is folded into x.
  - s-matmul is FLIPPED: out[64b, 16j] tiles = Rg[128n, 64b]^T @
    W3[128n, 16j], accumulating 32 (i, nt) steps per (cb, c8) into a
    PSUM region that is ALREADY the final [B, (c, j)] layout -- no
    transposes or per-capsule drains; one ACT copy per cb.
  - one [64, 512] f32 AllReduce per iteration, then a DVE-only squash
    (bit-trick rsqrt + 2 Newton steps; no ACT table switching -- the
    only ACT table used is exp_and_others).
t=0 (uniform couplings) is the same flipped matmul with x as lhsT and
W3 [128, (c8, j)] as rhs.  Logits are recomputed each iteration from
the running sum of outputs (the update is linear), so no [B,C,N] state.

Scheduling: the iteration boundary is pipelined per capsule-block --
s-matmul(t, cb) -> drain -> AllReduce(cb) -> squash(cb) -> osum(cb) ->
BD(g=cb) -> iteration t+1's beta units for group g=cb, so the next
iteration's beta phase overlaps the current s-phase.  The only global
barrier per iteration is the softmax normalizer Z (it needs all 32
capsules' exp tables).  PSUM budget is exactly 8 banks: 2x pA
[128,1024] + 2x beta^T [128,NT,128] + osumT + s-accumulator.
"""

import contextlib

import numpy as np

_nullctx = contextlib.nullcontext

B, N, I = 64, 4096, 8
C, D = 32, 16
ROUTINGS = 3
EPS = 1e-7
NCORES = 8
NL = N // NCORES          # 512 n per core
NT = NL // 128            # 4 partition tiles of n
CHUNKS = NL * I // 512    # 8 chunks of 512 along flat (n,i)


# ---------------------------------------------------------------------------
# Host-side layout prep (pure numpy, per core)
# ---------------------------------------------------------------------------

_CONSTS = {}


def _prep_consts():
    if not _CONSTS:
        import ml_dtypes
        _CONSTS["eyef"] = np.eye(128, dtype=np.float32)
        _CONSTS["eyeb"] = np.eye(128, dtype=np.float32).astype(ml_dtypes.bfloat16)
        _CONSTS["bdmask"] = _bd_mask().astype(ml_dtypes.bfloat16)
    return _CONSTS


def host_prep(x, W, k):
    """Per-core input layouts for core k (n slice [k*NL, (k+1)*NL))."""
    n0 = k * NL
    Wk = np.ascontiguousarray(W[:, n0:n0 + NL])          # [C, NL, D, I]
    xk = np.ascontiguousarray(x[:, n0:n0 + NL])          # [B, NL, I]

    # W2 [128=(cp*16+j), (cg, n*8+i)]  = W[cg*8+cp, n, j, i]   (bf16)
    w2 = Wk.reshape(4, 8, NL, D, I).transpose(1, 3, 0, 2, 4).reshape(128, 4 * NL * I)
    # W3 [128=nn, (cb, nt, i, c8, j)] = W[cb*8+c8, nt*128+nn, j, i]  (bf16)
    w3 = Wk.reshape(4, 8, NT, 128, D, I).transpose(3, 0, 2, 5, 1, 4) \
           .reshape(128, NT * I * C * D)
    # xt3 [128=nn, (i, nt, b)] = x[b, nt*128+nn, i]             (bf16)
    xt3 = xk.reshape(B, NT, 128, I).transpose(2, 3, 1, 0).reshape(128, I * NT * B)
    # xr2 [64=b, (n*8+i)] = x[b, n, i]  (bf16; device duplicates rows)
    xr2 = xk.reshape(B, NL * I)

    import ml_dtypes
    bf = ml_dtypes.bfloat16
    cst = _prep_consts()
    return {
        "w2": w2.astype(bf),
        "w3": w3.astype(bf),
        "xt3": xt3.astype(bf),
        "xr2": xr2.astype(bf),
        "eyef": cst["eyef"],
        "eyeb": cst["eyeb"],
        "bdmask": cst["bdmask"],
    }


def host_prep_all(x, W):
    """Vectorized host_prep for all cores at once."""
    import ml_dtypes
    bf = ml_dtypes.bfloat16
    Wb = np.ascontiguousarray(W, dtype=np.float32).astype(bf)   # [C, N, D, I]
    xb = np.ascontiguousarray(x, dtype=np.float32).astype(bf)   # [B, N, I]
    K = NCORES
    w2 = Wb.reshape(4, 8, K, NL, D, I).transpose(2, 1, 4, 0, 3, 5) \
           .reshape(K, 128, 4 * NL * I)
    w3 = Wb.reshape(4, 8, K, NT, 128, D, I).transpose(2, 4, 0, 3, 6, 1, 5) \
           .reshape(K, 128, NT * I * C * D)
    xt3 = xb.reshape(B, K, NT, 128, I).transpose(1, 3, 4, 2, 0) \
            .reshape(K, 128, I * NT * B)
    xr2 = xb.reshape(B, K, NL * I).transpose(1, 0, 2)           # [k, 64, 4096]
    cst = _prep_consts()
    return [
        {"w2": np.ascontiguousarray(w2[k]),
         "w3": np.ascontiguousarray(w3[k]),
         "xt3": np.ascontiguousarray(xt3[k]),
         "xr2": np.ascontiguousarray(xr2[k]),
         "eyef": cst["eyef"], "eyeb": cst["eyeb"], "bdmask": cst["bdmask"]}
        for k in range(K)
    ]


def _bd_mask():
    # mask[r, col] = 1 where ((r%32)//16) == col//64 — selects which b-half
    # of a block-diagonal lhsT tile each 16-row (one capsule's j-block) feeds.
    r = np.arange(128)[:, None]
    col = np.arange(128)[None, :]
    return (((r % 32) // 16) == (col // 64)).astype(np.float32)


# ---------------------------------------------------------------------------
# Bass device program
# ---------------------------------------------------------------------------

_CACHE = {}

# x-mul path per quarter slot (64 quarters/iter = 16 units x 4):
#   D = DVE direct from PSUM (f32 in, bf16 out, 1x)
#   V = ACT drain to bf16 + DVE mul (2x)
#   P = ACT drain to bf16 + Pool mul
# per-iteration pattern: the t=1 beta phase overlaps t0 (ACT busy with
# t0/xfer work -> D-heavier); the t=2 beta phase overlaps the t=1
# s-phase (DVE busy with Rg -> V-heavy)
QPATH_BY_T = {
    1: "D V V V V D V V V V D V V V V D".split(),
    2: "D V V V V D V V V V D V V V V D".split(),
}
# (cb, i) Rg-mul subunits assigned to Pool instead of DVE, per t
RG_POOL_T = {
    1: {(0, 3), (1, 3), (2, 3), (3, 3), (0, 6)},
    2: {(0, 3), (1, 3), (2, 3), (3, 3), (0, 6), (1, 6), (2, 6), (3, 6)},
}

MAGIC = 0x5F3759DF

# targeted scheduler-priority boosts (tested individually; blanket
# boosting everything regressed)
HP_FLUSHZ = False
HP_TAIL = False
HP_EXP = False
# emit Pool Rg-muls as two nt-half instructions (finer interleave)
POOL_RG_SPLIT = True
WP_BUFS = 2
RP_BUFS = 2
EXP_SPLIT = True


def _build_nc(sim=False):
    import concourse.bass as bass
    import concourse.bacc as bacc
    import concourse.mybir as mybir
    import concourse.tile as tile

    dt = mybir.dt
    f32, bf16, i32 = dt.float32, dt.bfloat16, dt.int32
    ALU = mybir.AluOpType
    AF = mybir.ActivationFunctionType
    AX = mybir.AxisListType

    nc = bacc.Bacc("TRN2", target_bir_lowering=False, debug=False,
                   num_devices=NCORES)

    w2_d = nc.dram_tensor("w2", [128, 4 * NL * I], bf16, kind="ExternalInput").ap()
    w3_d = nc.dram_tensor("w3", [128, NT * I * C * D], bf16, kind="ExternalInput").ap()
    xt3_d = nc.dram_tensor("xt3", [128, I * NT * B], bf16, kind="ExternalInput").ap()
    xr2_d = nc.dram_tensor("xr2", [B, NL * I], bf16, kind="ExternalInput").ap()
    eyef_d = nc.dram_tensor("eyef", [128, 128], f32, kind="ExternalInput").ap()
    eyeb_d = nc.dram_tensor("eyeb", [128, 128], bf16, kind="ExternalInput").ap()
    bdm_d = nc.dram_tensor("bdmask", [128, 128], bf16, kind="ExternalInput").ap()
    out_d = nc.dram_tensor("out", [B, C * D], f32, kind="ExternalOutput").ap()

    with tile.TileContext(nc) as tc:
        with (
            tc.tile_pool(name="const", bufs=1) as cp,
            tc.tile_pool(name="work", bufs=WP_BUFS) as wp,
            tc.tile_pool(name="rg", bufs=RP_BUFS) as rp,
            tc.tile_pool(name="dram", bufs=2, space="DRAM") as dp,
        ):
            sW2 = cp.tile([128, 4, CHUNKS, 512], bf16)
            sW3 = cp.tile([128, 4, NT, I, 8, D], bf16)
            sXT3 = cp.tile([128, I, NT, B], bf16)
            sXR2 = cp.tile([128, CHUNKS, 512], bf16)
            sEyeF = cp.tile([128, 128], f32)
            sEyeB = cp.tile([128, 128], bf16)
            sBdm = cp.tile([128, 128], bf16)

            # DMA-in: single queue in strict priority order — DMA transfers
            # serialize on the shared DMA-engine device, so arrival order is
            # consumption order: t0-cb0 inputs, then xr2/w2g0 (needed by the
            # first beta block), then alternating w3-cb / w2-g.
            w3v = sW3[:].rearrange("p cb a b c d -> p cb (a b c d)")
            qsz = NT * I * 8 * D
            w2v = sW2[:].rearrange("p g a b -> p g (a b)")
            gsz = CHUNKS * 512
            xrv = sXR2[:].rearrange("p a b -> p (a b)")
            nc.sync.dma_start(sXT3[:].rearrange("p a b c -> p (a b c)"), xt3_d[:])
            nc.sync.dma_start(sEyeB[:], eyeb_d[:])
            nc.sync.dma_start(sEyeF[:], eyef_d[:])
            nc.sync.dma_start(sBdm[:], bdm_d[:])
            ssz = qsz // 4
            for s_ in range(4):
                nc.sync.dma_start(w3v[:, 0, s_ * ssz:(s_ + 1) * ssz],
                                  w3_d[:, s_ * ssz:(s_ + 1) * ssz])
            nc.sync.dma_start(xrv[0:B, :], xr2_d[:])
            nc.sync.dma_start(xrv[B:128, :], xr2_d[:])
            nc.sync.dma_start(w2v[:, 0, :], w2_d[:, 0:gsz])
            for q_ in range(1, 4):
                nc.sync.dma_start(w3v[:, q_, :],
                                  w3_d[:, q_ * qsz:(q_ + 1) * qsz])
                nc.sync.dma_start(w2v[:, q_, :],
                                  w2_d[:, q_ * gsz:(q_ + 1) * gsz])

            sET = cp.tile([128, NT, C, B], bf16)
            sXt = cp.tile([128, I, NT, B], bf16)
            zA = cp.tile([128, NT, 16, B], bf16)
            zB = cp.tile([128, NT, 8, B], bf16)
            sZ = cp.tile([128, NT, B], bf16)
            sZr = cp.tile([128, NT, B], bf16)
            sS = cp.tile([B, C * D], f32)
            sSr = cp.tile([B, C * D], f32)
            sOut = cp.tile([B, C * D], f32)
            sOsum = cp.tile([B, C * D], f32)
            sOsumT = cp.tile([128, 4, B], bf16)
            sBDall = cp.tile([128, 16, 128], bf16)
            # squash temps (DVE-only; rsqrt via bit trick + Newton)
            sq = cp.tile([B, C * D], f32)
            s2 = cp.tile([B, C], f32)
            s2e = cp.tile([B, C], f32)
            ry = cp.tile([B, C], f32)
            rt = cp.tile([B, C], f32)
            opp = cp.tile([B, C], f32)
            rden = cp.tile([B, C], f32)
            fac = cp.tile([B, C], f32)

            nc.vector.memset(sBDall[:], 0.0)

            def squash_cb(src, dst, cb):
                sl = slice(cb * 128, (cb + 1) * 128)
                cs = slice(cb * 8, (cb + 1) * 8)
                nc.vector.tensor_mul(sq[:, sl], src[:, sl], src[:, sl])
                nc.vector.tensor_reduce(
                    s2[:, cs], sq[:, sl].rearrange("b (c j) -> b c j", j=D),
                    axis=AX.X, op=ALU.add)
                nc.vector.tensor_scalar_add(s2e[:, cs], s2[:, cs], EPS)
                # rsqrt(s2e): quake seed + 2 Newton steps (DVE-only, so ACT
                # never switches activation tables away from exp)
                yi = ry[:, cs].bitcast(i32)
                xi = s2e[:, cs].bitcast(i32)
                nc.vector.tensor_scalar(yi, xi, 1, None,
                                        op0=ALU.logical_shift_right)
                nc.vector.tensor_scalar(yi, yi, -1, MAGIC,
                                        op0=ALU.mult, op1=ALU.add)
                for _ in range(2):
                    nc.vector.tensor_mul(rt[:, cs], ry[:, cs], ry[:, cs])
                    nc.vector.tensor_mul(rt[:, cs], rt[:, cs], s2e[:, cs])
                    nc.vector.tensor_scalar(rt[:, cs], rt[:, cs], -0.5, 1.5,
                                            op0=ALU.mult, op1=ALU.add)
                    nc.vector.tensor_mul(ry[:, cs], ry[:, cs], rt[:, cs])
                nc.vector.tensor_scalar_add(opp[:, cs], s2[:, cs], 1.0)
                nc.vector.reciprocal(rden[:, cs], opp[:, cs])
                nc.vector.tensor_mul(fac[:, cs], s2[:, cs], ry[:, cs])
                nc.vector.tensor_mul(fac[:, cs], fac[:, cs], rden[:, cs])
                fb = fac[:, cs].rearrange("b (c o) -> b c o", o=1) \
                    .broadcast_to([B, 8, D])
                nc.vector.tensor_mul(
                    dst[:, sl].rearrange("b (c j) -> b c j", j=D),
                    src[:, sl].rearrange("b (c j) -> b c j", j=D), fb)

            def all_reduce_cb(cb):
                sl = slice(cb * 128, (cb + 1) * 128)
                if sim:
                    nc.vector.tensor_copy(sSr[:, sl], sS[:, sl])
                else:
                    di = dp.tile([B, 128], f32, tag="ar_in")
                    do = dp.tile([B, 128], f32, tag="ar_out")
                    nc.sync.dma_start(di[:], sS[:, sl])
                    nc.gpsimd.collective_compute(
                        "AllReduce", mybir.AluOpType.add,
                        replica_groups=[list(range(NCORES))],
                        ins=[di[:].opt()], outs=[do[:].opt()])
                    nc.sync.dma_start(sSr[:, sl], do[:])

            # --- pipelined schedule -----------------------------------
            # per capsule-block cb: s-matmul(t, cb) -> drain -> AllReduce
            # -> squash -> osum -> BD(g=cb) -> NEXT iteration's beta units
            # for g=cb.  The only global barrier per iteration is softmax Z.

            with (
                tc.tile_pool(name="psA", bufs=2, space="PSUM") as psA,
                tc.tile_pool(name="psT", bufs=2, space="PSUM") as psT,
                tc.tile_pool(name="psS", bufs=1, space="PSUM") as psS,
            ):
                pending = [None]

                def emit_reduce():
                    if pending[0] is None:
                        return
                    g, p, pT2, tmp = pending[0]
                    pending[0] = None
                    c0 = g * 8 + 2 * p
                    with (tc.high_priority() if HP_EXP
                          else _nullctx()):
                        if EXP_SPLIT:
                            for h_ in range(2):
                                ns = slice(2 * h_, 2 * h_ + 2)
                                nc.scalar.activation(
                                    sET[:, ns, c0:c0 + 2, :]
                                       .rearrange("p nt a b -> p nt (a b)"),
                                    pT2[:, ns, :]
                                       .rearrange("p nt f -> p (nt f)"),
                                    AF.Exp)
                        else:
                            nc.scalar.activation(
                                sET[:, :, c0:c0 + 2, :]
                                   .rearrange("p nt a b -> p nt (a b)"),
                                pT2[:].rearrange("p nt f -> p (nt f)"), AF.Exp)
                        if g >= 2:
                            # level-1 Z: e[c] + e[c+16] for this p's pair
                            cl = (g - 2) * 8 + 2 * p
                            nc.vector.tensor_add(
                                zA[:, :, cl:cl + 2, :],
                                sET[:, :, cl:cl + 2, :],
                                sET[:, :, cl + 16:cl + 18, :])

                def emit_traccum_q(q):
                    g, p, pT2, tmp = pending[0]
                    t8 = tmp[:, 2 * q:2 * q + 2, :] \
                        .rearrange("p a b -> p (a b)") \
                        .rearrange("p (n i) -> p n i", i=I)
                    for i in range(I):
                        nc.tensor.matmul(
                            pT2[:, q, :], t8[:, :, i], sEyeB[:],
                            start=(i == 0), stop=(i == I - 1))

                def emit_pb_unit(g, p, qpath):
                    # one beta unit (capsule pair) of group g
                    u = g * 4 + p
                    pT2 = psT.tile([128, NT, 128], f32, name="pT2",
                                   tag="bT")
                    tmp = wp.tile([128, CHUNKS, 512], bf16, name="tmp",
                                  tag="tmp")
                    for q in range(4):
                        pA = psA.tile([128, 1024], f32, name="pA",
                                      tag="pA")
                        for h in range(2):
                            nc.tensor.matmul(
                                pA[:, 512 * h:512 * (h + 1)],
                                sBDall[:, u, :],
                                sW2[:, g, 2 * q + h, :],
                                start=True, stop=True)
                        path = qpath[(u * 4 + q) % 16]
                        tv = tmp[:, 2 * q:2 * q + 2, :] \
                            .rearrange("p a b -> p (a b)")
                        xv = sXR2[:, 2 * q:2 * q + 2, :] \
                            .rearrange("p a b -> p (a b)")
                        if path == "D":
                            nc.vector.tensor_mul(tv, pA[:], xv)
                        else:
                            nc.scalar.copy(tv, pA[:])
                            meng = nc.gpsimd if path == "P" else nc.vector
                            meng.tensor_mul(tv, tv, xv)
                        # previous unit's quarter-q i-reduce fills PE
                        # while this unit's x-mul is still running
                        if pending[0] is not None:
                            emit_traccum_q(q)
                            if q == 3:
                                emit_reduce()
                    pending[0] = (g, p, pT2, tmp)

                def flush_z():
                    if pending[0] is not None:
                        for q in range(4):
                            emit_traccum_q(q)
                        emit_reduce()
                    hpz = tc.high_priority() if HP_FLUSHZ else _nullctx()
                    hpz.__enter__()
                    nc.vector.tensor_add(zB[:], zA[:, :, 0:8, :],
                                         zA[:, :, 8:16, :])
                    nc.vector.tensor_add(zA[:, :, 0:4, :], zB[:, :, 0:4, :],
                                         zB[:, :, 4:8, :])
                    nc.vector.tensor_add(zB[:, :, 0:2, :], zA[:, :, 0:2, :],
                                         zA[:, :, 2:4, :])
                    nc.vector.tensor_add(
                        sZ[:].rearrange("p nt (o b) -> p nt o b", o=1),
                        zB[:, :, 0:1, :], zB[:, :, 1:2, :])
                    with nc.allow_low_precision(reason="Z~32 in bf16"):
                        nc.vector.reciprocal(sZr[:], sZ[:])
                    for h in range(2):
                        nt0, nt1 = h * 2, h * 2 + 2
                        zb = sZr[:, nt0:nt1, :] \
                            .rearrange("p (o nt) b -> p o nt b", o=1) \
                            .broadcast_to([128, I, 2, B])
                        nc.vector.tensor_mul(sXt[:, :, nt0:nt1, :],
                                             sXT3[:, :, nt0:nt1, :], zb)
                    hpz.__exit__(None, None, None)

                slot_rgs = {}

                def emit_rg(t, cb, irange):
                    # coupling * x products feeding the cb s-matmul
                    rgs = slot_rgs.setdefault(cb, [None] * I)
                    for i in irange:
                        rg = rp.tile([128, NT, 8, B], bf16,
                                     name=f"rg{i}", tag=f"rg{i}")
                        xb = sXt[:, i, :, :] \
                            .rearrange("p nt (o b) -> p nt o b", o=1) \
                            .broadcast_to([128, NT, 8, B])
                        pool = (cb, i) in RG_POOL_T[t]
                        if pool and POOL_RG_SPLIT:
                            for h_ in range(2):
                                ns = slice(2 * h_, 2 * h_ + 2)
                                nc.gpsimd.tensor_mul(
                                    rg[:, ns, :, :],
                                    sET[:, ns, cb * 8:(cb + 1) * 8, :],
                                    xb[:, ns, :, :])
                        else:
                            meng = nc.gpsimd if pool else nc.vector
                            meng.tensor_mul(
                                rg[:], sET[:, :, cb * 8:(cb + 1) * 8, :], xb)
                        rgs[i] = rg

                def emit_smm(t, cb):
                    # s-matmul for capsule block cb at routing step t
                    pS = psS.tile([B, 128], f32, name="pS", tag="sS")
                    if t == 0:
                        # nt-outer so cb0's first w3 sub-chunk DMA unblocks
                        # the first steps
                        step = 0
                        for nt in range(NT):
                            for i in range(I):
                                rhs = sW3[:, cb, nt, i, :, :] \
                                    .rearrange("p a b -> p (a b)")
                                nc.tensor.matmul(
                                    pS[:], sXT3[:, i, nt, :], rhs,
                                    start=(step == 0), stop=(step == 31))
                                step += 1
                        nc.scalar.mul(sS[:, cb * 128:(cb + 1) * 128], pS[:],
                                      1.0 / C)
                        return
                    rgs = slot_rgs.pop(cb)
                    # one accumulation group open per PSUM tile at a time
                    for c8 in range(8):
                        for i in range(I):
                            for nt in range(NT):
                                nc.tensor.matmul(
                                    pS[:, c8 * D:(c8 + 1) * D],
                                    rgs[i][:, nt, c8, :],
                                    sW3[:, cb, nt, i, c8, :],
                                    start=(i == 0 and nt == 0),
                                    stop=(i == I - 1 and nt == NT - 1))
                    nc.scalar.copy(sS[:, cb * 128:(cb + 1) * 128], pS[:])

                def emit_pd(t, cb):
                    if t > 0:
                        emit_rg(t, cb, range(I))
                    emit_smm(t, cb)

                def emit_tail(t, cb):
                    last = (t == ROUTINGS - 1)
                    hpt = tc.high_priority() if HP_TAIL else _nullctx()
                    hpt.__enter__()
                    all_reduce_cb(cb)
                    sl = slice(cb * 128, (cb + 1) * 128)
                    if t == 0:
                        squash_cb(sSr, sOsum, cb)
                    else:
                        squash_cb(sSr, sOut, cb)
                        if not last:
                            nc.vector.tensor_add(sOsum[:, sl], sOsum[:, sl],
                                                 sOut[:, sl])
                    if last:
                        nc.sync.dma_start(out_d[:, sl], sOut[:, sl])
                        hpt.__exit__(None, None, None)
                        return
                    # transpose osum block, build BD tiles for group g=cb
                    pT = psT.tile([128, 4, B], f32, name="pT", tag="ot",
                                  bufs=1)
                    nc.tensor.transpose(pT[:, cb, :], sOsum[:, sl],
                                        sEyeF[0:B, 0:B])
                    nc.scalar.copy(sOsumT[:, cb, :], pT[:, cb, :])
                    g = cb
                    for p in range(4):
                        ob = sOsumT[32 * p:32 * p + 32, g, :] \
                            .rearrange("p (o b) -> p o b", o=1) \
                            .broadcast_to([32, 2, B])
                        nc.vector.tensor_mul(
                            sBDall[32 * p:32 * p + 32, g * 4 + p, :]
                                .rearrange("p (h b) -> p h b", h=2),
                            ob,
                            sBdm[32 * p:32 * p + 32, :]
                                .rearrange("p (h b) -> p h b", h=2))
                    hpt.__exit__(None, None, None)

                for t in range(ROUTINGS):
                    emit_pd(t, 0)
                    for cb in range(4):
                        emit_tail(t, cb)
                        if t < ROUTINGS - 1:
                            # next iteration's beta units for g=cb, with the
                            # next cb's Rg/s-matmul interleaved so no engine
                            # queue sees head-of-line blocking
                            nxt = cb + 1 if cb < 3 else None
                            for p in range(4):
                                emit_pb_unit(g=cb, p=p,
                                             qpath=QPATH_BY_T[t + 1])
                            if nxt is not None:
                                if t > 0:
                                    emit_rg(t, nxt, range(I))
                                emit_smm(t, nxt)
                        elif cb < 3:
                            emit_pd(t, cb + 1)
                    if t < ROUTINGS - 1:
                        flush_z()
    nc.compile()
    return nc


def get_nc(sim=False):
    key = "nc_sim" if sim else "nc"
    if key not in _CACHE:
        _CACHE[key] = _build_nc(sim=sim)
    return _CACHE[key]


def kernel(inputs, W):
    inputs = np.asarray(inputs, dtype=np.float32)
    W = np.asarray(W, dtype=np.float32)
    nc = get_nc()
    in_maps = host_prep_all(inputs, W)
    from concourse import bass_utils
    res = bass_utils.run_bass_kernel_spmd(
        nc, in_maps, core_ids=list(range(NCORES)))
    return res.results[0]["out"].reshape(B, C, D).astype(np.float32)


# revision 42
# speedup vs baseline: 1.0788x; 1.0039x over previous
"""CapsuleLayer dynamic-routing kernel for 8 Trainium2 NeuronCores.

Problem: inputs [64, 4096, 8] f32, W [32, 4096, 16, 8] f32.
  hat[b,c,n,j] = sum_i W[c,n,j,i] * x[b,n,i]
  3 routing iterations: c = softmax_C(b); out = squash(sum_n c*hat);
  b += <out, hat>.

Strategy: shard the n (input-capsule) axis across the 8 cores
(N_loc = 512/core); everything stays SBUF-resident; hat is never
materialized.  Per routing iteration t>0:
  - logits: A[(c2,b),(n,i)] = sum_j out*W via block-diagonal matmuls
    (BD lhsT built from transposed osum each iteration), then an
    elementwise multiply by x (split across DVE-direct-from-PSUM /
    ACT-drain+DVE-bf16 / ACT-drain+Pool paths, tunable), and the
    i-reduction is folded into PE transpose-accumulation: 8 strided
    [128,128] transposes of the product accumulate in PSUM, yielding
    beta^T [128n, (c2,b)] directly (f32 accumulation, no add tree).
  - exp straight from PSUM into the e-table (ACT); softmax Z is a
    bf16 pairwise add tree on DVE whose first level is emitted
    per-unit-pair during the beta phase; 1/Z is folded into x.
  - s-matmul is FLIPPED: out[64b, 16j] tiles = Rg[128n, 64b]^T @
    W3[128n, 16j], accumulating 32 (i, nt) steps per (cb, c8) into a
    PSUM region that is ALREADY the final [B, (c, j)] layout -- no
    transposes or per-capsule drains; one ACT copy per cb.
  - one [64, 512] f32 AllReduce per iteration, then a DVE-only squash
    (bit-trick rsqrt + 2 Newton steps; no ACT table switching -- the
    only ACT table used is exp_and_others).
t=0 (uniform couplings) is the same flipped matmul with x as lhsT and
W3 [128, (c8, j)] as rhs.  Logits are recomputed each iteration from
the running sum of outputs (the update is linear), so no [B,C,N] state.

Scheduling: the iteration boundary is pipelined per capsule-block --
s-matmul(t, cb) -> drain -> AllReduce(cb) -> squash(cb) -> osum(cb) ->
BD(g=cb) -> iteration t+1's beta units for group g=cb, so the next
iteration's beta phase overlaps the current s-phase.  The only global
barrier per iteration is the softmax normalizer Z (it needs all 32
capsules' exp tables).  PSUM budget is exactly 8 banks: 2x pA
[128,1024] + 2x beta^T [128,NT,128] + osumT + s-accumulator.
"""

import contextlib

import numpy as np

_nullctx = contextlib.nullcontext

B, N, I = 64, 4096, 8
C, D = 32, 16
ROUTINGS = 3
EPS = 1e-7
NCORES = 8
NL = N // NCORES          # 512 n per core
NT = NL // 128            # 4 partition tiles of n
CHUNKS = NL * I // 512    # 8 chunks of 512 along flat (n,i)


# ---------------------------------------------------------------------------
# Host-side layout prep (pure numpy, per core)
# ---------------------------------------------------------------------------

_CONSTS = {}


def _prep_consts():
    if not _CONSTS:
        import ml_dtypes
        _CONSTS["eyef"] = np.eye(128, dtype=np.float32)
        _CONSTS["eyeb"] = np.eye(128, dtype=np.float32).astype(ml_dtypes.bfloat16)
        _CONSTS["bdmask"] = _bd_mask().astype(ml_dtypes.bfloat16)
    return _CONSTS


def host_prep(x, W, k):
    """Per-core input layouts for core k (n slice [k*NL, (k+1)*NL))."""
    n0 = k * NL
    Wk = np.ascontiguousarray(W[:, n0:n0 + NL])          # [C, NL, D, I]
    xk = np.ascontiguousarray(x[:, n0:n0 + NL])          # [B, NL, I]

    # W2 [128=(cp*16+j), (cg, n*8+i)]  = W[cg*8+cp, n, j, i]   (bf16)
    w2 = Wk.reshape(4, 8, NL, D, I).transpose(1, 3, 0, 2, 4).reshape(128, 4 * NL * I)
    # W3 [128=nn, (cb, nt, i, c8, j)] = W[cb*8+c8, nt*128+nn, j, i]  (bf16)
    w3 = Wk.reshape(4, 8, NT, 128, D, I).transpose(3, 0, 2, 5, 1, 4) \
           .reshape(128, NT * I * C * D)
    # xt3 [128=nn, (i, nt, b)] = x[b, nt*128+nn, i]             (bf16)
    xt3 = xk.reshape(B, NT, 128, I).transpose(2, 3, 1, 0).reshape(128, I * NT * B)
    # xr2 [64=b, (n*8+i)] = x[b, n, i]  (bf16; device duplicates rows)
    xr2 = xk.reshape(B, NL * I)

    import ml_dtypes
    bf = ml_dtypes.bfloat16
    cst = _prep_consts()
    return {
        "w2": w2.astype(bf),
        "w3": w3.astype(bf),
        "xt3": xt3.astype(bf),
        "xr2": xr2.astype(bf),
        "eyef": cst["eyef"],
        "eyeb": cst["eyeb"],
        "bdmask": cst["bdmask"],
    }


def host_prep_all(x, W):
    """Vectorized host_prep for all cores at once."""
    import ml_dtypes
    bf = ml_dtypes.bfloat16
    Wb = np.ascontiguousarray(W, dtype=np.float32).astype(bf)   # [C, N, D, I]
    xb = np.ascontiguousarray(x, dtype=np.float32).astype(bf)   # [B, N, I]
    K = NCORES
    w2 = Wb.reshape(4, 8, K, NL, D, I).transpose(2, 1, 4, 0, 3, 5) \
           .reshape(K, 128, 4 * NL * I)
    w3 = Wb.reshape(4, 8, K, NT, 128, D, I).transpose(2, 4, 0, 3, 6, 1, 5) \
           .reshape(K, 128, NT * I * C * D)
    xt3 = xb.reshape(B, K, NT, 128, I).transpose(1, 3, 4, 2, 0) \
            .reshape(K, 128, I * NT * B)
    xr2 = xb.reshape(B, K, NL * I).transpose(1, 0, 2)           # [k, 64, 4096]
    cst = _prep_consts()
    return [
        {"w2": np.ascontiguousarray(w2[k]),
         "w3": np.ascontiguousarray(w3[k]),
         "xt3": np.ascontiguousarray(xt3[k]),
         "xr2": np.ascontiguousarray(xr2[k]),
         "eyef": cst["eyef"], "eyeb": cst["eyeb"], "bdmask": cst["bdmask"]}
        for k in range(K)
    ]


def _bd_mask():
    # mask[r, col] = 1 where ((r%32)//16) == col//64 — selects which b-half
    # of a block-diagonal lhsT tile each 16-row (one capsule's j-block) feeds.
    r = np.arange(128)[:, None]
    col = np.arange(128)[None, :]
    return (((r % 32) // 16) == (col // 64)).astype(np.float32)


# ---------------------------------------------------------------------------
# Bass device program
# ---------------------------------------------------------------------------

_CACHE = {}

# x-mul path per quarter slot (64 quarters/iter = 16 units x 4):
#   D = DVE direct from PSUM (f32 in, bf16 out, 1x)
#   V = ACT drain to bf16 + DVE mul (2x)
#   P = ACT drain to bf16 + Pool mul
# per-iteration pattern: the t=1 beta phase overlaps t0 (ACT busy with
# t0/xfer work -> D-heavier); the t=2 beta phase overlaps the t=1
# s-phase (DVE busy with Rg -> V-heavy)
QPATH_BY_T = {
    1: "D V D V V D V V D V D V V V V D".split(),
    2: "D V V V V D V V V V D V V V V D".split(),
}
# (cb, i) Rg-mul subunits assigned to Pool instead of DVE, per t
RG_POOL_T = {
    1: {(0, 3), (1, 3), (2, 3), (3, 3), (0, 6)},
    2: {(0, 3), (1, 3), (2, 3), (3, 3), (0, 6), (1, 6), (2, 6), (3, 6)},
}

MAGIC = 0x5F3759DF

# targeted scheduler-priority boosts (tested individually; blanket
# boosting everything regressed)
HP_FLUSHZ = False
HP_TAIL = False
HP_EXP = False
# emit Pool Rg-muls as two nt-half instructions (finer interleave)
POOL_RG_SPLIT = True
WP_BUFS = 2
RP_BUFS = 2
EXP_SPLIT = True
# drain-free DVE work moved to the otherwise-idle Pool engine
L1_POOL = False
# merge DVE Rg-muls into i-pairs (fewer per-instruction overheads)
RG_PAIR = False
BD_POOL = True
OSUM_POOL = True


def _build_nc(sim=False):
    import concourse.bass as bass
    import concourse.bacc as bacc
    import concourse.mybir as mybir
    import concourse.tile as tile

    dt = mybir.dt
    f32, bf16, i32 = dt.float32, dt.bfloat16, dt.int32
    ALU = mybir.AluOpType
    AF = mybir.ActivationFunctionType
    AX = mybir.AxisListType

    nc = bacc.Bacc("TRN2", target_bir_lowering=False, debug=False,
                   num_devices=NCORES)

    w2_d = nc.dram_tensor("w2", [128, 4 * NL * I], bf16, kind="ExternalInput").ap()
    w3_d = nc.dram_tensor("w3", [128, NT * I * C * D], bf16, kind="ExternalInput").ap()
    xt3_d = nc.dram_tensor("xt3", [128, I * NT * B], bf16, kind="ExternalInput").ap()
    xr2_d = nc.dram_tensor("xr2", [B, NL * I], bf16, kind="ExternalInput").ap()
    eyef_d = nc.dram_tensor("eyef", [128, 128], f32, kind="ExternalInput").ap()
    eyeb_d = nc.dram_tensor("eyeb", [128, 128], bf16, kind="ExternalInput").ap()
    bdm_d = nc.dram_tensor("bdmask", [128, 128], bf16, kind="ExternalInput").ap()
    out_d = nc.dram_tensor("out", [B, C * D], f32, kind="ExternalOutput").ap()

    with tile.TileContext(nc) as tc:
        with (
            tc.tile_pool(name="const", bufs=1) as cp,
            tc.tile_pool(name="work", bufs=WP_BUFS) as wp,
            tc.tile_pool(name="rg", bufs=RP_BUFS) as rp,
            tc.tile_pool(name="dram", bufs=2, space="DRAM") as dp,
        ):
            sW2 = cp.tile([128, 4, CHUNKS, 512], bf16)
            sW3 = cp.tile([128, 4, NT, I, 8, D], bf16)
            sXT3 = cp.tile([128, I, NT, B], bf16)
            sXR2 = cp.tile([128, CHUNKS, 512], bf16)
            sEyeF = cp.tile([128, 128], f32)
            sEyeB = cp.tile([128, 128], bf16)
            sBdm = cp.tile([128, 128], bf16)

            # DMA-in: single queue in strict priority order — DMA transfers
            # serialize on the shared DMA-engine device, so arrival order is
            # consumption order: t0-cb0 inputs, then xr2/w2g0 (needed by the
            # first beta block), then alternating w3-cb / w2-g.
            w3v = sW3[:].rearrange("p cb a b c d -> p cb (a b c d)")
            qsz = NT * I * 8 * D
            w2v = sW2[:].rearrange("p g a b -> p g (a b)")
            gsz = CHUNKS * 512
            xrv = sXR2[:].rearrange("p a b -> p (a b)")
            nc.sync.dma_start(sXT3[:].rearrange("p a b c -> p (a b c)"), xt3_d[:])
            nc.sync.dma_start(sEyeB[:], eyeb_d[:])
            nc.sync.dma_start(sEyeF[:], eyef_d[:])
            nc.sync.dma_start(sBdm[:], bdm_d[:])
            ssz = qsz // 4
            for s_ in range(4):
                nc.sync.dma_start(w3v[:, 0, s_ * ssz:(s_ + 1) * ssz],
                                  w3_d[:, s_ * ssz:(s_ + 1) * ssz])
            nc.sync.dma_start(xrv[0:B, :], xr2_d[:])
            nc.sync.dma_start(xrv[B:128, :], xr2_d[:])
            nc.sync.dma_start(w2v[:, 0, :], w2_d[:, 0:gsz])
            for q_ in range(1, 4):
                nc.sync.dma_start(w3v[:, q_, :],
                                  w3_d[:, q_ * qsz:(q_ + 1) * qsz])
                nc.sync.dma_start(w2v[:, q_, :],
                                  w2_d[:, q_ * gsz:(q_ + 1) * gsz])

            sET = cp.tile([128, NT, C, B], bf16)
            sXt = cp.tile([128, I, NT, B], bf16)
            zA = cp.tile([128, NT, 16, B], bf16)
            zB = cp.tile([128, NT, 8, B], bf16)
            sZ = cp.tile([128, NT, B], bf16)
            sZr = cp.tile([128, NT, B], bf16)
            sS = cp.tile([B, C * D], f32)
            sSr = cp.tile([B, C * D], f32)
            sOut = cp.tile([B, C * D], f32)
            sOsum = cp.tile([B, C * D], f32)
            sOsumT = cp.tile([128, 4, B], bf16)
            sBDall = cp.tile([128, 16, 128], bf16)
            # squash temps (DVE-only; rsqrt via bit trick + Newton)
            sq = cp.tile([B, C * D], f32)
            s2 = cp.tile([B, C], f32)
            s2e = cp.tile([B, C], f32)
            ry = cp.tile([B, C], f32)
            rt = cp.tile([B, C], f32)
            opp = cp.tile([B, C], f32)
            rden = cp.tile([B, C], f32)
            fac = cp.tile([B, C], f32)

            nc.vector.memset(sBDall[:], 0.0)

            def squash_cb(src, dst, cb):
                sl = slice(cb * 128, (cb + 1) * 128)
                cs = slice(cb * 8, (cb + 1) * 8)
                nc.vector.tensor_mul(sq[:, sl], src[:, sl], src[:, sl])
                nc.vector.tensor_reduce(
                    s2[:, cs], sq[:, sl].rearrange("b (c j) -> b c j", j=D),
                    axis=AX.X, op=ALU.add)
                nc.vector.tensor_scalar_add(s2e[:, cs], s2[:, cs], EPS)
                # rsqrt(s2e): quake seed + 2 Newton steps (DVE-only, so ACT
                # never switches activation tables away from exp)
                yi = ry[:, cs].bitcast(i32)
                xi = s2e[:, cs].bitcast(i32)
                nc.vector.tensor_scalar(yi, xi, 1, None,
                                        op0=ALU.logical_shift_right)
                nc.vector.tensor_scalar(yi, yi, -1, MAGIC,
                                        op0=ALU.mult, op1=ALU.add)
                for _ in range(2):
                    nc.vector.tensor_mul(rt[:, cs], ry[:, cs], ry[:, cs])
                    nc.vector.tensor_mul(rt[:, cs], rt[:, cs], s2e[:, cs])
                    nc.vector.tensor_scalar(rt[:, cs], rt[:, cs], -0.5, 1.5,
                                            op0=ALU.mult, op1=ALU.add)
                    nc.vector.tensor_mul(ry[:, cs], ry[:, cs], rt[:, cs])
                nc.vector.tensor_scalar_add(opp[:, cs], s2[:, cs], 1.0)
                nc.vector.reciprocal(rden[:, cs], opp[:, cs])
                nc.vector.tensor_mul(fac[:, cs], s2[:, cs], ry[:, cs])
                nc.vector.tensor_mul(fac[:, cs], fac[:, cs], rden[:, cs])
                fb = fac[:, cs].rearrange("b (c o) -> b c o", o=1) \
                    .broadcast_to([B, 8, D])
                nc.vector.tensor_mul(
                    dst[:, sl].rearrange("b (c j) -> b c j", j=D),
                    src[:, sl].rearrange("b (c j) -> b c j", j=D), fb)

            def all_reduce_cb(cb):
                sl = slice(cb * 128, (cb + 1) * 128)
                if sim:
                    nc.vector.tensor_copy(sSr[:, sl], sS[:, sl])
                else:
                    di = dp.tile([B, 128], f32, tag="ar_in")
                    do = dp.tile([B, 128], f32, tag="ar_out")
                    nc.sync.dma_start(di[:], sS[:, sl])
                    nc.gpsimd.collective_compute(
                        "AllReduce", mybir.AluOpType.add,
                        replica_groups=[list(range(NCORES))],
                        ins=[di[:].opt()], outs=[do[:].opt()])
                    nc.sync.dma_start(sSr[:, sl], do[:])

            # --- pipelined schedule -----------------------------------
            # per capsule-block cb: s-matmul(t, cb) -> drain -> AllReduce
            # -> squash -> osum -> BD(g=cb) -> NEXT iteration's beta units
            # for g=cb.  The only global barrier per iteration is softmax Z.

            with (
                tc.tile_pool(name="psA", bufs=2, space="PSUM") as psA,
                tc.tile_pool(name="psT", bufs=2, space="PSUM") as psT,
                tc.tile_pool(name="psS", bufs=1, space="PSUM") as psS,
            ):
                pending = [None]

                def emit_reduce():
                    if pending[0] is None:
                        return
                    g, p, pT2, tmp = pending[0]
                    pending[0] = None
                    c0 = g * 8 + 2 * p
                    with (tc.high_priority() if HP_EXP
                          else _nullctx()):
                        if EXP_SPLIT:
                            for h_ in range(2):
                                ns = slice(2 * h_, 2 * h_ + 2)
                                nc.scalar.activation(
                                    sET[:, ns, c0:c0 + 2, :]
                                       .rearrange("p nt a b -> p nt (a b)"),
                                    pT2[:, ns, :]
                                       .rearrange("p nt f -> p (nt f)"),
                                    AF.Exp)
                        else:
                            nc.scalar.activation(
                                sET[:, :, c0:c0 + 2, :]
                                   .rearrange("p nt a b -> p nt (a b)"),
                                pT2[:].rearrange("p nt f -> p (nt f)"), AF.Exp)
                        if g >= 2:
                            # level-1 Z: e[c] + e[c+16] for this p's pair
                            cl = (g - 2) * 8 + 2 * p
                            zeng = nc.gpsimd if L1_POOL else nc.vector
                            zeng.tensor_add(
                                zA[:, :, cl:cl + 2, :],
                                sET[:, :, cl:cl + 2, :],
                                sET[:, :, cl + 16:cl + 18, :])

                def emit_traccum_q(q):
                    g, p, pT2, tmp = pending[0]
                    t8 = tmp[:, 2 * q:2 * q + 2, :] \
                        .rearrange("p a b -> p (a b)") \
                        .rearrange("p (n i) -> p n i", i=I)
                    for i in range(I):
                        nc.tensor.matmul(
                            pT2[:, q, :], t8[:, :, i], sEyeB[:],
                            start=(i == 0), stop=(i == I - 1))

                def emit_pb_unit(g, p, qpath):
                    # one beta unit (capsule pair) of group g
                    u = g * 4 + p
                    pT2 = psT.tile([128, NT, 128], f32, name="pT2",
                                   tag="bT")
                    tmp = wp.tile([128, CHUNKS, 512], bf16, name="tmp",
                                  tag="tmp")
                    for q in range(4):
                        pA = psA.tile([128, 1024], f32, name="pA",
                                      tag="pA")
                        for h in range(2):
                            nc.tensor.matmul(
                                pA[:, 512 * h:512 * (h + 1)],
                                sBDall[:, u, :],
                                sW2[:, g, 2 * q + h, :],
                                start=True, stop=True)
                        path = qpath[(u * 4 + q) % 16]
                        tv = tmp[:, 2 * q:2 * q + 2, :] \
                            .rearrange("p a b -> p (a b)")
                        xv = sXR2[:, 2 * q:2 * q + 2, :] \
                            .rearrange("p a b -> p (a b)")
                        if path == "D":
                            nc.vector.tensor_mul(tv, pA[:], xv)
                        else:
                            nc.scalar.copy(tv, pA[:])
                            meng = nc.gpsimd if path == "P" else nc.vector
                            meng.tensor_mul(tv, tv, xv)
                        # previous unit's quarter-q i-reduce fills PE
                        # while this unit's x-mul is still running
                        if pending[0] is not None:
                            emit_traccum_q(q)
                            if q == 3:
                                emit_reduce()
                    pending[0] = (g, p, pT2, tmp)

                def flush_z():
                    if pending[0] is not None:
                        for q in range(4):
                            emit_traccum_q(q)
                        emit_reduce()
                    hpz = tc.high_priority() if HP_FLUSHZ else _nullctx()
                    hpz.__enter__()
                    nc.vector.tensor_add(zB[:], zA[:, :, 0:8, :],
                                         zA[:, :, 8:16, :])
                    nc.vector.tensor_add(zA[:, :, 0:4, :], zB[:, :, 0:4, :],
                                         zB[:, :, 4:8, :])
                    nc.vector.tensor_add(zB[:, :, 0:2, :], zA[:, :, 0:2, :],
                                         zA[:, :, 2:4, :])
                    nc.vector.tensor_add(
                        sZ[:].rearrange("p nt (o b) -> p nt o b", o=1),
                        zB[:, :, 0:1, :], zB[:, :, 1:2, :])
                    with nc.allow_low_precision(reason="Z~32 in bf16"):
                        nc.vector.reciprocal(sZr[:], sZ[:])
                    for h in range(2):
                        nt0, nt1 = h * 2, h * 2 + 2
                        zb = sZr[:, nt0:nt1, :] \
                            .rearrange("p (o nt) b -> p o nt b", o=1) \
                            .broadcast_to([128, I, 2, B])
                        nc.vector.tensor_mul(sXt[:, :, nt0:nt1, :],
                                             sXT3[:, :, nt0:nt1, :], zb)
                    hpz.__exit__(None, None, None)

                slot_rgs = {}

                def emit_rg(t, cb, irange):
                    # coupling * x products feeding the cb s-matmul
                    rgs = slot_rgs.setdefault(cb, [None] * I)
                    done = set()
                    for i in irange:
                        if i in done:
                            continue
                        pool = (cb, i) in RG_POOL_T[t]
                        j = i + 1
                        pairable = (RG_PAIR and not pool and j in irange
                                    and (cb, j) not in RG_POOL_T[t])
                        if pairable:
                            rg = rp.tile([128, 2, NT, 8, B], bf16,
                                         name=f"rgp{i}", tag=f"rg{i}")
                            x2 = sXt[:, i:i + 2, :, :] \
                                .rearrange("p i nt (o b) -> p i nt o b", o=1) \
                                .broadcast_to([128, 2, NT, 8, B])
                            e2 = sET[:, :, cb * 8:(cb + 1) * 8, :] \
                                .rearrange("p (o nt) c b -> p o nt c b", o=1) \
                                .broadcast_to([128, 2, NT, 8, B])
                            nc.vector.tensor_mul(rg[:], e2, x2)
                            rgs[i] = rg[:, 0]
                            rgs[j] = rg[:, 1]
                            done.add(j)
                            continue
                        rg = rp.tile([128, NT, 8, B], bf16,
                                     name=f"rg{i}", tag=f"rg{i}")
                        xb = sXt[:, i, :, :] \
                            .rearrange("p nt (o b) -> p nt o b", o=1) \
                            .broadcast_to([128, NT, 8, B])
                        if pool and POOL_RG_SPLIT:
                            for h_ in range(2):
                                ns = slice(2 * h_, 2 * h_ + 2)
                                nc.gpsimd.tensor_mul(
                                    rg[:, ns, :, :],
                                    sET[:, ns, cb * 8:(cb + 1) * 8, :],
                                    xb[:, ns, :, :])
                        else:
                            meng = nc.gpsimd if pool else nc.vector
                            meng.tensor_mul(
                                rg[:], sET[:, :, cb * 8:(cb + 1) * 8, :], xb)
                        rgs[i] = rg

                def emit_smm(t, cb):
                    # s-matmul for capsule block cb at routing step t
                    pS = psS.tile([B, 128], f32, name="pS", tag="sS")
                    if t == 0:
                        # nt-outer so cb0's first w3 sub-chunk DMA unblocks
                        # the first steps
                        step = 0
                        for nt in range(NT):
                            for i in range(I):
                                rhs = sW3[:, cb, nt, i, :, :] \
                                    .rearrange("p a b -> p (a b)")
                                nc.tensor.matmul(
                                    pS[:], sXT3[:, i, nt, :], rhs,
                                    start=(step == 0), stop=(step == 31))
                                step += 1
                        nc.scalar.mul(sS[:, cb * 128:(cb + 1) * 128], pS[:],
                                      1.0 / C)
                        return
                    rgs = slot_rgs.pop(cb)
                    # one accumulation group open per PSUM tile at a time
                    for c8 in range(8):
                        for i in range(I):
                            for nt in range(NT):
                                nc.tensor.matmul(
                                    pS[:, c8 * D:(c8 + 1) * D],
                                    rgs[i][:, nt, c8, :],
                                    sW3[:, cb, nt, i, c8, :],
                                    start=(i == 0 and nt == 0),
                                    stop=(i == I - 1 and nt == NT - 1))
                    nc.scalar.copy(sS[:, cb * 128:(cb + 1) * 128], pS[:])

                def emit_pd(t, cb):
                    if t > 0:
                        emit_rg(t, cb, range(I))
                    emit_smm(t, cb)

                def emit_tail(t, cb):
                    last = (t == ROUTINGS - 1)
                    hpt = tc.high_priority() if HP_TAIL else _nullctx()
                    hpt.__enter__()
                    all_reduce_cb(cb)
                    sl = slice(cb * 128, (cb + 1) * 128)
                    if t == 0:
                        squash_cb(sSr, sOsum, cb)
                    else:
                        squash_cb(sSr, sOut, cb)
                        if not last:
                            oeng = nc.gpsimd if OSUM_POOL else nc.vector
                            oeng.tensor_add(sOsum[:, sl], sOsum[:, sl],
                                            sOut[:, sl])
                    if last:
                        nc.sync.dma_start(out_d[:, sl], sOut[:, sl])
                        hpt.__exit__(None, None, None)
                        return
                    # transpose osum block, build BD tiles for group g=cb
                    pT = psT.tile([128, 4, B], f32, name="pT", tag="ot",
                                  bufs=1)
                    nc.tensor.transpose(pT[:, cb, :], sOsum[:, sl],
                                        sEyeF[0:B, 0:B])
                    nc.scalar.copy(sOsumT[:, cb, :], pT[:, cb, :])
                    g = cb
                    beng = nc.gpsimd if BD_POOL else nc.vector
                    for p in range(4):
                        ob = sOsumT[32 * p:32 * p + 32, g, :] \
                            .rearrange("p (o b) -> p o b", o=1) \
                            .broadcast_to([32, 2, B])
                        beng.tensor_mul(
                            sBDall[32 * p:32 * p + 32, g * 4 + p, :]
                                .rearrange("p (h b) -> p h b", h=2),
                            ob,
                            sBdm[32 * p:32 * p + 32, :]
                                .rearrange("p (h b) -> p h b", h=2))
                    hpt.__exit__(None, None, None)

                for t in range(ROUTINGS):
                    emit_pd(t, 0)
                    for cb in range(4):
                        emit_tail(t, cb)
                        if t < ROUTINGS - 1:
                            # next iteration's beta units for g=cb, with the
                            # next cb's Rg/s-matmul interleaved so no engine
                            # queue sees head-of-line blocking
                            nxt = cb + 1 if cb < 3 else None
                            for p in range(4):
                                emit_pb_unit(g=cb, p=p,
                                             qpath=QPATH_BY_T[t + 1])
                            if nxt is not None:
                                if t > 0:
                                    emit_rg(t, nxt, range(I))
                                emit_smm(t, nxt)
                        elif cb < 3:
                            emit_pd(t, cb + 1)
                    if t < ROUTINGS - 1:
                        flush_z()
    nc.compile()
    return nc


def get_nc(sim=False):
    key = "nc_sim" if sim else "nc"
    if key not in _CACHE:
        _CACHE[key] = _build_nc(sim=sim)
    return _CACHE[key]


def kernel(inputs, W):
    inputs = np.asarray(inputs, dtype=np.float32)
    W = np.asarray(W, dtype=np.float32)
    nc = get_nc()
    in_maps = host_prep_all(inputs, W)
    from concourse import bass_utils
    res = bass_utils.run_bass_kernel_spmd(
        nc, in_maps, core_ids=list(range(NCORES)))
    return res.results[0]["out"].reshape(B, C, D).astype(np.float32)


# revision 43
# speedup vs baseline: 1.0802x; 1.0013x over previous
"""CapsuleLayer dynamic-routing kernel for 8 Trainium2 NeuronCores.

Problem: inputs [64, 4096, 8] f32, W [32, 4096, 16, 8] f32.
  hat[b,c,n,j] = sum_i W[c,n,j,i] * x[b,n,i]
  3 routing iterations: c = softmax_C(b); out = squash(sum_n c*hat);
  b += <out, hat>.

Strategy: shard the n (input-capsule) axis across the 8 cores
(N_loc = 512/core); everything stays SBUF-resident; hat is never
materialized.  Per routing iteration t>0:
  - logits: A[(c2,b),(n,i)] = sum_j out*W via block-diagonal matmuls
    (BD lhsT built from transposed osum each iteration), then an
    elementwise multiply by x (split across DVE-direct-from-PSUM /
    ACT-drain+DVE-bf16 / ACT-drain+Pool paths, tunable), and the
    i-reduction is folded into PE transpose-accumulation: 8 strided
    [128,128] transposes of the product accumulate in PSUM, yielding
    beta^T [128n, (c2,b)] directly (f32 accumulation, no add tree).
  - exp straight from PSUM into the e-table (ACT); softmax Z is a
    bf16 pairwise add tree on DVE whose first level is emitted
    per-unit-pair during the beta phase; 1/Z is folded into x.
  - s-matmul is FLIPPED: out[64b, 16j] tiles = Rg[128n, 64b]^T @
    W3[128n, 16j], accumulating 32 (i, nt) steps per (cb, c8) into a
    PSUM region that is ALREADY the final [B, (c, j)] layout -- no
    transposes or per-capsule drains; one ACT copy per cb.
  - one [64, 512] f32 AllReduce per iteration, then a DVE-only squash
    (bit-trick rsqrt + 2 Newton steps; no ACT table switching -- the
    only ACT table used is exp_and_others).
t=0 (uniform couplings) is the same flipped matmul with x as lhsT and
W3 [128, (c8, j)] as rhs.  Logits are recomputed each iteration from
the running sum of outputs (the update is linear), so no [B,C,N] state.

Scheduling: the iteration boundary is pipelined per capsule-block --
s-matmul(t, cb) -> drain -> AllReduce(cb) -> squash(cb) -> osum(cb) ->
BD(g=cb) -> iteration t+1's beta units for group g=cb, so the next
iteration's beta phase overlaps the current s-phase.  The only global
barrier per iteration is the softmax normalizer Z (it needs all 32
capsules' exp tables).  PSUM budget is exactly 8 banks: 2x pA
[128,1024] + 2x beta^T [128,NT,128] + osumT + s-accumulator.
"""

import contextlib

import numpy as np

_nullctx = contextlib.nullcontext

B, N, I = 64, 4096, 8
C, D = 32, 16
ROUTINGS = 3
EPS = 1e-7
NCORES = 8
NL = N // NCORES          # 512 n per core
NT = NL // 128            # 4 partition tiles of n
CHUNKS = NL * I // 512    # 8 chunks of 512 along flat (n,i)


# ---------------------------------------------------------------------------
# Host-side layout prep (pure numpy, per core)
# ---------------------------------------------------------------------------

_CONSTS = {}


def _prep_consts():
    if not _CONSTS:
        import ml_dtypes
        _CONSTS["eyef"] = np.eye(128, dtype=np.float32)
        _CONSTS["eyeb"] = np.eye(128, dtype=np.float32).astype(ml_dtypes.bfloat16)
        _CONSTS["bdmask"] = _bd_mask().astype(ml_dtypes.bfloat16)
    return _CONSTS


def host_prep(x, W, k):
    """Per-core input layouts for core k (n slice [k*NL, (k+1)*NL))."""
    n0 = k * NL
    Wk = np.ascontiguousarray(W[:, n0:n0 + NL])          # [C, NL, D, I]
    xk = np.ascontiguousarray(x[:, n0:n0 + NL])          # [B, NL, I]

    # W2 [128=(cp*16+j), (cg, n*8+i)]  = W[cg*8+cp, n, j, i]   (bf16)
    w2 = Wk.reshape(4, 8, NL, D, I).transpose(1, 3, 0, 2, 4).reshape(128, 4 * NL * I)
    # W3 [128=nn, (cb, nt, i, c8, j)] = W[cb*8+c8, nt*128+nn, j, i]  (bf16)
    w3 = Wk.reshape(4, 8, NT, 128, D, I).transpose(3, 0, 2, 5, 1, 4) \
           .reshape(128, NT * I * C * D)
    # xt3 [128=nn, (i, nt, b)] = x[b, nt*128+nn, i]             (bf16)
    xt3 = xk.reshape(B, NT, 128, I).transpose(2, 3, 1, 0).reshape(128, I * NT * B)
    # xr2 [64=b, (n*8+i)] = x[b, n, i]  (bf16; device duplicates rows)
    xr2 = xk.reshape(B, NL * I)

    import ml_dtypes
    bf = ml_dtypes.bfloat16
    cst = _prep_consts()
    return {
        "w2": w2.astype(bf),
        "w3": w3.astype(bf),
        "xt3": xt3.astype(bf),
        "xr2": xr2.astype(bf),
        "eyef": cst["eyef"],
        "eyeb": cst["eyeb"],
        "bdmask": cst["bdmask"],
    }


def host_prep_all(x, W):
    """Vectorized host_prep for all cores at once."""
    import ml_dtypes
    bf = ml_dtypes.bfloat16
    Wb = np.ascontiguousarray(W, dtype=np.float32).astype(bf)   # [C, N, D, I]
    xb = np.ascontiguousarray(x, dtype=np.float32).astype(bf)   # [B, N, I]
    K = NCORES
    w2 = Wb.reshape(4, 8, K, NL, D, I).transpose(2, 1, 4, 0, 3, 5) \
           .reshape(K, 128, 4 * NL * I)
    w3 = Wb.reshape(4, 8, K, NT, 128, D, I).transpose(2, 4, 0, 3, 6, 1, 5) \
           .reshape(K, 128, NT * I * C * D)
    xt3 = xb.reshape(B, K, NT, 128, I).transpose(1, 3, 4, 2, 0) \
            .reshape(K, 128, I * NT * B)
    xr2 = xb.reshape(B, K, NL * I).transpose(1, 0, 2)           # [k, 64, 4096]
    cst = _prep_consts()
    return [
        {"w2": np.ascontiguousarray(w2[k]),
         "w3": np.ascontiguousarray(w3[k]),
         "xt3": np.ascontiguousarray(xt3[k]),
         "xr2": np.ascontiguousarray(xr2[k]),
         "eyef": cst["eyef"], "eyeb": cst["eyeb"], "bdmask": cst["bdmask"]}
        for k in range(K)
    ]


def _bd_mask():
    # mask[r, col] = 1 where ((r%32)//16) == col//64 — selects which b-half
    # of a block-diagonal lhsT tile each 16-row (one capsule's j-block) feeds.
    r = np.arange(128)[:, None]
    col = np.arange(128)[None, :]
    return (((r % 32) // 16) == (col // 64)).astype(np.float32)


# ---------------------------------------------------------------------------
# Bass device program
# ---------------------------------------------------------------------------

_CACHE = {}

# x-mul path per quarter slot (64 quarters/iter = 16 units x 4):
#   D = DVE direct from PSUM (f32 in, bf16 out, 1x)
#   V = ACT drain to bf16 + DVE mul (2x)
#   P = ACT drain to bf16 + Pool mul
# per-iteration pattern: the t=1 beta phase overlaps t0 (ACT busy with
# t0/xfer work -> D-heavier); the t=2 beta phase overlaps the t=1
# s-phase (DVE busy with Rg -> V-heavy)
QPATH_BY_T = {
    1: "D V D V V D V D D V D V V V V D".split(),
    2: "D V V V V D V V V V D V V V V D".split(),
}
# (cb, i) Rg-mul subunits assigned to Pool instead of DVE, per t
RG_POOL_T = {
    1: {(0, 3), (1, 3), (2, 3), (3, 3), (0, 6)},
    2: {(0, 3), (1, 3), (2, 3), (3, 3), (0, 6), (1, 6), (2, 6), (3, 6)},
}

MAGIC = 0x5F3759DF

# targeted scheduler-priority boosts (tested individually; blanket
# boosting everything regressed)
HP_FLUSHZ = False
HP_TAIL = False
HP_EXP = False
# emit Pool Rg-muls as two nt-half instructions (finer interleave)
POOL_RG_SPLIT = True
WP_BUFS = 2
RP_BUFS = 2
EXP_SPLIT = True
# drain-free DVE work moved to the otherwise-idle Pool engine
L1_POOL = False
# merge DVE Rg-muls into i-pairs (fewer per-instruction overheads)
RG_PAIR = False
BD_POOL = True
OSUM_POOL = True


def _build_nc(sim=False):
    import concourse.bass as bass
    import concourse.bacc as bacc
    import concourse.mybir as mybir
    import concourse.tile as tile

    dt = mybir.dt
    f32, bf16, i32 = dt.float32, dt.bfloat16, dt.int32
    ALU = mybir.AluOpType
    AF = mybir.ActivationFunctionType
    AX = mybir.AxisListType

    nc = bacc.Bacc("TRN2", target_bir_lowering=False, debug=False,
                   num_devices=NCORES)

    w2_d = nc.dram_tensor("w2", [128, 4 * NL * I], bf16, kind="ExternalInput").ap()
    w3_d = nc.dram_tensor("w3", [128, NT * I * C * D], bf16, kind="ExternalInput").ap()
    xt3_d = nc.dram_tensor("xt3", [128, I * NT * B], bf16, kind="ExternalInput").ap()
    xr2_d = nc.dram_tensor("xr2", [B, NL * I], bf16, kind="ExternalInput").ap()
    eyef_d = nc.dram_tensor("eyef", [128, 128], f32, kind="ExternalInput").ap()
    eyeb_d = nc.dram_tensor("eyeb", [128, 128], bf16, kind="ExternalInput").ap()
    bdm_d = nc.dram_tensor("bdmask", [128, 128], bf16, kind="ExternalInput").ap()
    out_d = nc.dram_tensor("out", [B, C * D], f32, kind="ExternalOutput").ap()

    with tile.TileContext(nc) as tc:
        with (
            tc.tile_pool(name="const", bufs=1) as cp,
            tc.tile_pool(name="work", bufs=WP_BUFS) as wp,
            tc.tile_pool(name="rg", bufs=RP_BUFS) as rp,
            tc.tile_pool(name="dram", bufs=2, space="DRAM") as dp,
        ):
            sW2 = cp.tile([128, 4, CHUNKS, 512], bf16)
            sW3 = cp.tile([128, 4, NT, I, 8, D], bf16)
            sXT3 = cp.tile([128, I, NT, B], bf16)
            sXR2 = cp.tile([128, CHUNKS, 512], bf16)
            sEyeF = cp.tile([128, 128], f32)
            sEyeB = cp.tile([128, 128], bf16)
            sBdm = cp.tile([128, 128], bf16)

            # DMA-in: single queue in strict priority order — DMA transfers
            # serialize on the shared DMA-engine device, so arrival order is
            # consumption order: t0-cb0 inputs, then xr2/w2g0 (needed by the
            # first beta block), then alternating w3-cb / w2-g.
            w3v = sW3[:].rearrange("p cb a b c d -> p cb (a b c d)")
            qsz = NT * I * 8 * D
            w2v = sW2[:].rearrange("p g a b -> p g (a b)")
            gsz = CHUNKS * 512
            xrv = sXR2[:].rearrange("p a b -> p (a b)")
            nc.sync.dma_start(sXT3[:].rearrange("p a b c -> p (a b c)"), xt3_d[:])
            nc.sync.dma_start(sEyeB[:], eyeb_d[:])
            nc.sync.dma_start(sEyeF[:], eyef_d[:])
            nc.sync.dma_start(sBdm[:], bdm_d[:])
            ssz = qsz // 4
            for s_ in range(4):
                nc.sync.dma_start(w3v[:, 0, s_ * ssz:(s_ + 1) * ssz],
                                  w3_d[:, s_ * ssz:(s_ + 1) * ssz])
            nc.sync.dma_start(xrv[0:B, :], xr2_d[:])
            nc.sync.dma_start(xrv[B:128, :], xr2_d[:])
            nc.sync.dma_start(w2v[:, 0, :], w2_d[:, 0:gsz])
            for q_ in range(1, 4):
                nc.sync.dma_start(w3v[:, q_, :],
                                  w3_d[:, q_ * qsz:(q_ + 1) * qsz])
                nc.sync.dma_start(w2v[:, q_, :],
                                  w2_d[:, q_ * gsz:(q_ + 1) * gsz])

            sET = cp.tile([128, NT, C, B], bf16)
            sXt = cp.tile([128, I, NT, B], bf16)
            zA = cp.tile([128, NT, 16, B], bf16)
            zB = cp.tile([128, NT, 8, B], bf16)
            sZ = cp.tile([128, NT, B], bf16)
            sZr = cp.tile([128, NT, B], bf16)
            sS = cp.tile([B, C * D], f32)
            sSr = cp.tile([B, C * D], f32)
            sOut = cp.tile([B, C * D], f32)
            sOsum = cp.tile([B, C * D], f32)
            sOsumT = cp.tile([128, 4, B], bf16)
            sBDall = cp.tile([128, 16, 128], bf16)
            # squash temps (DVE-only; rsqrt via bit trick + Newton)
            sq = cp.tile([B, C * D], f32)
            s2 = cp.tile([B, C], f32)
            s2e = cp.tile([B, C], f32)
            ry = cp.tile([B, C], f32)
            rt = cp.tile([B, C], f32)
            opp = cp.tile([B, C], f32)
            rden = cp.tile([B, C], f32)
            fac = cp.tile([B, C], f32)

            nc.vector.memset(sBDall[:], 0.0)

            def squash_cb(src, dst, cb):
                sl = slice(cb * 128, (cb + 1) * 128)
                cs = slice(cb * 8, (cb + 1) * 8)
                nc.vector.tensor_mul(sq[:, sl], src[:, sl], src[:, sl])
                nc.vector.tensor_reduce(
                    s2[:, cs], sq[:, sl].rearrange("b (c j) -> b c j", j=D),
                    axis=AX.X, op=ALU.add)
                nc.vector.tensor_scalar_add(s2e[:, cs], s2[:, cs], EPS)
                # rsqrt(s2e): quake seed + 2 Newton steps (DVE-only, so ACT
                # never switches activation tables away from exp)
                yi = ry[:, cs].bitcast(i32)
                xi = s2e[:, cs].bitcast(i32)
                nc.vector.tensor_scalar(yi, xi, 1, None,
                                        op0=ALU.logical_shift_right)
                nc.vector.tensor_scalar(yi, yi, -1, MAGIC,
                                        op0=ALU.mult, op1=ALU.add)
                for _ in range(2):
                    nc.vector.tensor_mul(rt[:, cs], ry[:, cs], ry[:, cs])
                    nc.vector.tensor_mul(rt[:, cs], rt[:, cs], s2e[:, cs])
                    nc.vector.tensor_scalar(rt[:, cs], rt[:, cs], -0.5, 1.5,
                                            op0=ALU.mult, op1=ALU.add)
                    nc.vector.tensor_mul(ry[:, cs], ry[:, cs], rt[:, cs])
                nc.vector.tensor_scalar_add(opp[:, cs], s2[:, cs], 1.0)
                nc.vector.reciprocal(rden[:, cs], opp[:, cs])
                nc.vector.tensor_mul(fac[:, cs], s2[:, cs], ry[:, cs])
                nc.vector.tensor_mul(fac[:, cs], fac[:, cs], rden[:, cs])
                fb = fac[:, cs].rearrange("b (c o) -> b c o", o=1) \
                    .broadcast_to([B, 8, D])
                nc.vector.tensor_mul(
                    dst[:, sl].rearrange("b (c j) -> b c j", j=D),
                    src[:, sl].rearrange("b (c j) -> b c j", j=D), fb)

            def all_reduce_cb(cb):
                sl = slice(cb * 128, (cb + 1) * 128)
                if sim:
                    nc.vector.tensor_copy(sSr[:, sl], sS[:, sl])
                else:
                    di = dp.tile([B, 128], f32, tag="ar_in")
                    do = dp.tile([B, 128], f32, tag="ar_out")
                    nc.sync.dma_start(di[:], sS[:, sl])
                    nc.gpsimd.collective_compute(
                        "AllReduce", mybir.AluOpType.add,
                        replica_groups=[list(range(NCORES))],
                        ins=[di[:].opt()], outs=[do[:].opt()])
                    nc.sync.dma_start(sSr[:, sl], do[:])

            # --- pipelined schedule -----------------------------------
            # per capsule-block cb: s-matmul(t, cb) -> drain -> AllReduce
            # -> squash -> osum -> BD(g=cb) -> NEXT iteration's beta units
            # for g=cb.  The only global barrier per iteration is softmax Z.

            with (
                tc.tile_pool(name="psA", bufs=2, space="PSUM") as psA,
                tc.tile_pool(name="psT", bufs=2, space="PSUM") as psT,
                tc.tile_pool(name="psS", bufs=1, space="PSUM") as psS,
            ):
                pending = [None]

                def emit_reduce():
                    if pending[0] is None:
                        return
                    g, p, pT2, tmp = pending[0]
                    pending[0] = None
                    c0 = g * 8 + 2 * p
                    with (tc.high_priority() if HP_EXP
                          else _nullctx()):
                        if EXP_SPLIT:
                            for h_ in range(2):
                                ns = slice(2 * h_, 2 * h_ + 2)
                                nc.scalar.activation(
                                    sET[:, ns, c0:c0 + 2, :]
                                       .rearrange("p nt a b -> p nt (a b)"),
                                    pT2[:, ns, :]
                                       .rearrange("p nt f -> p (nt f)"),
                                    AF.Exp)
                        else:
                            nc.scalar.activation(
                                sET[:, :, c0:c0 + 2, :]
                                   .rearrange("p nt a b -> p nt (a b)"),
                                pT2[:].rearrange("p nt f -> p (nt f)"), AF.Exp)
                        if g >= 2:
                            # level-1 Z: e[c] + e[c+16] for this p's pair
                            cl = (g - 2) * 8 + 2 * p
                            zeng = nc.gpsimd if L1_POOL else nc.vector
                            zeng.tensor_add(
                                zA[:, :, cl:cl + 2, :],
                                sET[:, :, cl:cl + 2, :],
                                sET[:, :, cl + 16:cl + 18, :])

                def emit_traccum_q(q):
                    g, p, pT2, tmp = pending[0]
                    t8 = tmp[:, 2 * q:2 * q + 2, :] \
                        .rearrange("p a b -> p (a b)") \
                        .rearrange("p (n i) -> p n i", i=I)
                    for i in range(I):
                        nc.tensor.matmul(
                            pT2[:, q, :], t8[:, :, i], sEyeB[:],
                            start=(i == 0), stop=(i == I - 1))

                def emit_pb_unit(g, p, qpath):
                    # one beta unit (capsule pair) of group g
                    u = g * 4 + p
                    pT2 = psT.tile([128, NT, 128], f32, name="pT2",
                                   tag="bT")
                    tmp = wp.tile([128, CHUNKS, 512], bf16, name="tmp",
                                  tag="tmp")
                    for q in range(4):
                        pA = psA.tile([128, 1024], f32, name="pA",
                                      tag="pA")
                        for h in range(2):
                            nc.tensor.matmul(
                                pA[:, 512 * h:512 * (h + 1)],
                                sBDall[:, u, :],
                                sW2[:, g, 2 * q + h, :],
                                start=True, stop=True)
                        path = qpath[(u * 4 + q) % 16]
                        tv = tmp[:, 2 * q:2 * q + 2, :] \
                            .rearrange("p a b -> p (a b)")
                        xv = sXR2[:, 2 * q:2 * q + 2, :] \
                            .rearrange("p a b -> p (a b)")
                        if path == "D":
                            nc.vector.tensor_mul(tv, pA[:], xv)
                        else:
                            nc.scalar.copy(tv, pA[:])
                            meng = nc.gpsimd if path == "P" else nc.vector
                            meng.tensor_mul(tv, tv, xv)
                        # previous unit's quarter-q i-reduce fills PE
                        # while this unit's x-mul is still running
                        if pending[0] is not None:
                            emit_traccum_q(q)
                            if q == 3:
                                emit_reduce()
                    pending[0] = (g, p, pT2, tmp)

                def flush_z():
                    if pending[0] is not None:
                        for q in range(4):
                            emit_traccum_q(q)
                        emit_reduce()
                    hpz = tc.high_priority() if HP_FLUSHZ else _nullctx()
                    hpz.__enter__()
                    nc.vector.tensor_add(zB[:], zA[:, :, 0:8, :],
                                         zA[:, :, 8:16, :])
                    nc.vector.tensor_add(zA[:, :, 0:4, :], zB[:, :, 0:4, :],
                                         zB[:, :, 4:8, :])
                    nc.vector.tensor_add(zB[:, :, 0:2, :], zA[:, :, 0:2, :],
                                         zA[:, :, 2:4, :])
                    nc.vector.tensor_add(
                        sZ[:].rearrange("p nt (o b) -> p nt o b", o=1),
                        zB[:, :, 0:1, :], zB[:, :, 1:2, :])
                    with nc.allow_low_precision(reason="Z~32 in bf16"):
                        nc.vector.reciprocal(sZr[:], sZ[:])
                    for h in range(2):
                        nt0, nt1 = h * 2, h * 2 + 2
                        zb = sZr[:, nt0:nt1, :] \
                            .rearrange("p (o nt) b -> p o nt b", o=1) \
                            .broadcast_to([128, I, 2, B])
                        nc.vector.tensor_mul(sXt[:, :, nt0:nt1, :],
                                             sXT3[:, :, nt0:nt1, :], zb)
                    hpz.__exit__(None, None, None)

                slot_rgs = {}

                def emit_rg(t, cb, irange):
                    # coupling * x products feeding the cb s-matmul
                    rgs = slot_rgs.setdefault(cb, [None] * I)
                    done = set()
                    for i in irange:
                        if i in done:
                            continue
                        pool = (cb, i) in RG_POOL_T[t]
                        j = i + 1
                        pairable = (RG_PAIR and not pool and j in irange
                                    and (cb, j) not in RG_POOL_T[t])
                        if pairable:
                            rg = rp.tile([128, 2, NT, 8, B], bf16,
                                         name=f"rgp{i}", tag=f"rg{i}")
                            x2 = sXt[:, i:i + 2, :, :] \
                                .rearrange("p i nt (o b) -> p i nt o b", o=1) \
                                .broadcast_to([128, 2, NT, 8, B])
                            e2 = sET[:, :, cb * 8:(cb + 1) * 8, :] \
                                .rearrange("p (o nt) c b -> p o nt c b", o=1) \
                                .broadcast_to([128, 2, NT, 8, B])
                            nc.vector.tensor_mul(rg[:], e2, x2)
                            rgs[i] = rg[:, 0]
                            rgs[j] = rg[:, 1]
                            done.add(j)
                            continue
                        rg = rp.tile([128, NT, 8, B], bf16,
                                     name=f"rg{i}", tag=f"rg{i}")
                        xb = sXt[:, i, :, :] \
                            .rearrange("p nt (o b) -> p nt o b", o=1) \
                            .broadcast_to([128, NT, 8, B])
                        if pool and POOL_RG_SPLIT:
                            for h_ in range(2):
                                ns = slice(2 * h_, 2 * h_ + 2)
                                nc.gpsimd.tensor_mul(
                                    rg[:, ns, :, :],
                                    sET[:, ns, cb * 8:(cb + 1) * 8, :],
                                    xb[:, ns, :, :])
                        else:
                            meng = nc.gpsimd if pool else nc.vector
                            meng.tensor_mul(
                                rg[:], sET[:, :, cb * 8:(cb + 1) * 8, :], xb)
                        rgs[i] = rg

                def emit_smm(t, cb):
                    # s-matmul for capsule block cb at routing step t
                    pS = psS.tile([B, 128], f32, name="pS", tag="sS")
                    if t == 0:
                        # nt-outer so cb0's first w3 sub-chunk DMA unblocks
                        # the first steps
                        step = 0
                        for nt in range(NT):
                            for i in range(I):
                                rhs = sW3[:, cb, nt, i, :, :] \
                                    .rearrange("p a b -> p (a b)")
                                nc.tensor.matmul(
                                    pS[:], sXT3[:, i, nt, :], rhs,
                                    start=(step == 0), stop=(step == 31))
                                step += 1
                        nc.scalar.mul(sS[:, cb * 128:(cb + 1) * 128], pS[:],
                                      1.0 / C)
                        return
                    rgs = slot_rgs.pop(cb)
                    # one accumulation group open per PSUM tile at a time
                    for c8 in range(8):
                        for i in range(I):
                            for nt in range(NT):
                                nc.tensor.matmul(
                                    pS[:, c8 * D:(c8 + 1) * D],
                                    rgs[i][:, nt, c8, :],
                                    sW3[:, cb, nt, i, c8, :],
                                    start=(i == 0 and nt == 0),
                                    stop=(i == I - 1 and nt == NT - 1))
                    nc.scalar.copy(sS[:, cb * 128:(cb + 1) * 128], pS[:])

                def emit_pd(t, cb):
                    if t > 0:
                        emit_rg(t, cb, range(I))
                    emit_smm(t, cb)

                def emit_tail(t, cb):
                    last = (t == ROUTINGS - 1)
                    hpt = tc.high_priority() if HP_TAIL else _nullctx()
                    hpt.__enter__()
                    all_reduce_cb(cb)
                    sl = slice(cb * 128, (cb + 1) * 128)
                    if t == 0:
                        squash_cb(sSr, sOsum, cb)
                    else:
                        squash_cb(sSr, sOut, cb)
                        if not last:
                            oeng = nc.gpsimd if OSUM_POOL else nc.vector
                            oeng.tensor_add(sOsum[:, sl], sOsum[:, sl],
                                            sOut[:, sl])
                    if last:
                        nc.sync.dma_start(out_d[:, sl], sOut[:, sl])
                        hpt.__exit__(None, None, None)
                        return
                    # transpose osum block, build BD tiles for group g=cb
                    pT = psT.tile([128, 4, B], f32, name="pT", tag="ot",
                                  bufs=1)
                    nc.tensor.transpose(pT[:, cb, :], sOsum[:, sl],
                                        sEyeF[0:B, 0:B])
                    nc.scalar.copy(sOsumT[:, cb, :], pT[:, cb, :])
                    g = cb
                    beng = nc.gpsimd if BD_POOL else nc.vector
                    for p in range(4):
                        ob = sOsumT[32 * p:32 * p + 32, g, :] \
                            .rearrange("p (o b) -> p o b", o=1) \
                            .broadcast_to([32, 2, B])
                        beng.tensor_mul(
                            sBDall[32 * p:32 * p + 32, g * 4 + p, :]
                                .rearrange("p (h b) -> p h b", h=2),
                            ob,
                            sBdm[32 * p:32 * p + 32, :]
                                .rearrange("p (h b) -> p h b", h=2))
                    hpt.__exit__(None, None, None)

                for t in range(ROUTINGS):
                    emit_pd(t, 0)
                    for cb in range(4):
                        emit_tail(t, cb)
                        if t < ROUTINGS - 1:
                            # next iteration's beta units for g=cb, with the
                            # next cb's Rg/s-matmul interleaved so no engine
                            # queue sees head-of-line blocking
                            nxt = cb + 1 if cb < 3 else None
                            for p in range(4):
                                emit_pb_unit(g=cb, p=p,
                                             qpath=QPATH_BY_T[t + 1])
                            if nxt is not None:
                                if t > 0:
                                    emit_rg(t, nxt, range(I))
                                emit_smm(t, nxt)
                        elif cb < 3:
                            emit_pd(t, cb + 1)
                    if t < ROUTINGS - 1:
                        flush_z()
    nc.compile()
    return nc


def get_nc(sim=False):
    key = "nc_sim" if sim else "nc"
    if key not in _CACHE:
        _CACHE[key] = _build_nc(sim=sim)
    return _CACHE[key]


def kernel(inputs, W):
    inputs = np.asarray(inputs, dtype=np.float32)
    W = np.asarray(W, dtype=np.float32)
    nc = get_nc()
    in_maps = host_prep_all(inputs, W)
    from concourse import bass_utils
    res = bass_utils.run_bass_kernel_spmd(
        nc, in_maps, core_ids=list(range(NCORES)))
    return res.results[0]["out"].reshape(B, C, D).astype(np.float32)


# revision 45
# speedup vs baseline: 1.0996x; 1.0179x over previous
"""CapsuleLayer dynamic-routing kernel for 8 Trainium2 NeuronCores.

Problem: inputs [64, 4096, 8] f32, W [32, 4096, 16, 8] f32.
  hat[b,c,n,j] = sum_i W[c,n,j,i] * x[b,n,i]
  3 routing iterations: c = softmax_C(b); out = squash(sum_n c*hat);
  b += <out, hat>.

Strategy: shard the n (input-capsule) axis across the 8 cores
(N_loc = 512/core); everything stays SBUF-resident; hat is never
materialized.  Per routing iteration t>0:
  - logits: A[(c2,b),(n,i)] = sum_j out*W via block-diagonal matmuls
    (BD lhsT built from transposed osum each iteration), then an
    elementwise multiply by x (split across DVE-direct-from-PSUM /
    ACT-drain+DVE-bf16 / ACT-drain+Pool paths, tunable), and the
    i-reduction is folded into PE transpose-accumulation: 8 strided
    [128,128] transposes of the product accumulate in PSUM, yielding
    beta^T [128n, (c2,b)] directly (f32 accumulation, no add tree).
  - exp straight from PSUM into the e-table (ACT); softmax Z is a
    bf16 pairwise add tree on DVE whose first level is emitted
    per-unit-pair during the beta phase; 1/Z is folded into x.
  - s-matmul is FLIPPED: out[64b, 16j] tiles = Rg[128n, 64b]^T @
    W3[128n, 16j], accumulating 32 (i, nt) steps per (cb, c8) into a
    PSUM region that is ALREADY the final [B, (c, j)] layout -- no
    transposes or per-capsule drains; one ACT copy per cb.
  - one [64, 512] f32 AllReduce per iteration, then a DVE-only squash
    (bit-trick rsqrt + 2 Newton steps; no ACT table switching -- the
    only ACT table used is exp_and_others).
t=0 (uniform couplings) is the same flipped matmul with x as lhsT and
W3 [128, (c8, j)] as rhs.  Logits are recomputed each iteration from
the running sum of outputs (the update is linear), so no [B,C,N] state.

Scheduling: the iteration boundary is pipelined per capsule-block --
s-matmul(t, cb) -> drain -> AllReduce(cb) -> squash(cb) -> osum(cb) ->
BD(g=cb) -> iteration t+1's beta units for group g=cb, so the next
iteration's beta phase overlaps the current s-phase.  The only global
barrier per iteration is the softmax normalizer Z (it needs all 32
capsules' exp tables).  PSUM budget is exactly 8 banks: 2x pA
[128,1024] + 2x beta^T [128,NT,128] + osumT + s-accumulator.
"""

import contextlib

import numpy as np

_nullctx = contextlib.nullcontext

B, N, I = 64, 4096, 8
C, D = 32, 16
ROUTINGS = 3
EPS = 1e-7
NCORES = 8
NL = N // NCORES          # 512 n per core
NT = NL // 128            # 4 partition tiles of n
CHUNKS = NL * I // 512    # 8 chunks of 512 along flat (n,i)


# ---------------------------------------------------------------------------
# Host-side layout prep (pure numpy, per core)
# ---------------------------------------------------------------------------

_CONSTS = {}


def _prep_consts():
    if not _CONSTS:
        import ml_dtypes
        _CONSTS["eyef"] = np.eye(128, dtype=np.float32)
        _CONSTS["eyeb"] = np.eye(128, dtype=np.float32).astype(ml_dtypes.bfloat16)
        _CONSTS["bdmask"] = _bd_mask().astype(ml_dtypes.bfloat16)
    return _CONSTS


def host_prep(x, W, k):
    """Per-core input layouts for core k (n slice [k*NL, (k+1)*NL))."""
    n0 = k * NL
    Wk = np.ascontiguousarray(W[:, n0:n0 + NL])          # [C, NL, D, I]
    xk = np.ascontiguousarray(x[:, n0:n0 + NL])          # [B, NL, I]

    # W2 [128=(cp*16+j), (cg, n*8+i)]  = W[cg*8+cp, n, j, i]   (bf16)
    w2 = Wk.reshape(4, 8, NL, D, I).transpose(1, 3, 0, 2, 4).reshape(128, 4 * NL * I)
    # W3 [128=nn, (cb, nt, i, c8, j)] = W[cb*8+c8, nt*128+nn, j, i]  (bf16)
    w3 = Wk.reshape(4, 8, NT, 128, D, I).transpose(3, 0, 2, 5, 1, 4) \
           .reshape(128, NT * I * C * D)
    # xt3 [128=nn, (i, nt, b)] = x[b, nt*128+nn, i]             (bf16)
    xt3 = xk.reshape(B, NT, 128, I).transpose(2, 3, 1, 0).reshape(128, I * NT * B)
    # xr2 [64=b, (n*8+i)] = x[b, n, i]  (bf16; device duplicates rows)
    xr2 = xk.reshape(B, NL * I)

    import ml_dtypes
    bf = ml_dtypes.bfloat16
    cst = _prep_consts()
    return {
        "w2": w2.astype(bf),
        "w3": w3.astype(bf),
        "xt3": xt3.astype(bf),
        "xr2": xr2.astype(bf),
        "eyef": cst["eyef"],
        "eyeb": cst["eyeb"],
        "bdmask": cst["bdmask"],
    }


def host_prep_all(x, W):
    """Vectorized host_prep for all cores at once."""
    import ml_dtypes
    bf = ml_dtypes.bfloat16
    Wb = np.ascontiguousarray(W, dtype=np.float32).astype(bf)   # [C, N, D, I]
    xb = np.ascontiguousarray(x, dtype=np.float32).astype(bf)   # [B, N, I]
    K = NCORES
    w2 = Wb.reshape(4, 8, K, NL, D, I).transpose(2, 1, 4, 0, 3, 5) \
           .reshape(K, 128, 4 * NL * I)
    w3 = Wb.reshape(4, 8, K, NT, 128, D, I).transpose(2, 4, 0, 3, 6, 1, 5) \
           .reshape(K, 128, NT * I * C * D)
    xt3 = xb.reshape(B, K, NT, 128, I).transpose(1, 3, 4, 2, 0) \
            .reshape(K, 128, I * NT * B)
    xr2 = xb.reshape(B, K, NL * I).transpose(1, 0, 2)           # [k, 64, 4096]
    cst = _prep_consts()
    return [
        {"w2": np.ascontiguousarray(w2[k]),
         "w3": np.ascontiguousarray(w3[k]),
         "xt3": np.ascontiguousarray(xt3[k]),
         "xr2": np.ascontiguousarray(xr2[k]),
         "eyef": cst["eyef"], "eyeb": cst["eyeb"], "bdmask": cst["bdmask"]}
        for k in range(K)
    ]


def _bd_mask():
    # mask[r, col] = 1 where ((r%32)//16) == col//64 — selects which b-half
    # of a block-diagonal lhsT tile each 16-row (one capsule's j-block) feeds.
    r = np.arange(128)[:, None]
    col = np.arange(128)[None, :]
    return (((r % 32) // 16) == (col // 64)).astype(np.float32)


# ---------------------------------------------------------------------------
# Bass device program
# ---------------------------------------------------------------------------

_CACHE = {}

# x-mul path per quarter slot (64 quarters/iter = 16 units x 4):
#   D = DVE direct from PSUM (f32 in, bf16 out, 1x)
#   V = ACT drain to bf16 + DVE mul (2x)
#   P = ACT drain to bf16 + Pool mul
# per-iteration pattern: the t=1 beta phase overlaps t0 (ACT busy with
# t0/xfer work -> D-heavier); the t=2 beta phase overlaps the t=1
# s-phase (DVE busy with Rg -> V-heavy)
QPATH_BY_T = {
    1: "D V D V V D V D D V D V V V V D".split(),
    2: "D V V V V D V V V V D V V V V D".split(),
}
# (cb, i) Rg-mul subunits assigned to Pool instead of DVE, per t
RG_POOL_T = {
    1: {(0, 3), (1, 3), (2, 3), (3, 3), (0, 6)},
    2: {(0, 3), (1, 3), (2, 3), (3, 3), (0, 6), (1, 6), (2, 6), (3, 6)},
}

MAGIC = 0x5F3759DF

# targeted scheduler-priority boosts (tested individually; blanket
# boosting everything regressed)
HP_FLUSHZ = False
HP_TAIL = False
HP_EXP = False
# emit Pool Rg-muls as two nt-half instructions (finer interleave)
POOL_RG_SPLIT = True
WP_BUFS = 2
RP_BUFS = 2
EXP_SPLIT = True
# drain-free DVE work moved to the otherwise-idle Pool engine
L1_POOL = False
# squash rsqrt Newton steps (1 is plenty: seed err 3.4e-2 -> 1.7e-3,
# output gate is 2e-2) and whether to add EPS (s2 >> eps always here)
NEWTON_STEPS = 1
SQUASH_EPS = True
# merge DVE Rg-muls into i-pairs (fewer per-instruction overheads)
RG_PAIR = False
BD_POOL = True
OSUM_POOL = True


def _build_nc(sim=False):
    import concourse.bass as bass
    import concourse.bacc as bacc
    import concourse.mybir as mybir
    import concourse.tile as tile

    dt = mybir.dt
    f32, bf16, i32 = dt.float32, dt.bfloat16, dt.int32
    ALU = mybir.AluOpType
    AF = mybir.ActivationFunctionType
    AX = mybir.AxisListType

    nc = bacc.Bacc("TRN2", target_bir_lowering=False, debug=False,
                   num_devices=NCORES)

    w2_d = nc.dram_tensor("w2", [128, 4 * NL * I], bf16, kind="ExternalInput").ap()
    w3_d = nc.dram_tensor("w3", [128, NT * I * C * D], bf16, kind="ExternalInput").ap()
    xt3_d = nc.dram_tensor("xt3", [128, I * NT * B], bf16, kind="ExternalInput").ap()
    xr2_d = nc.dram_tensor("xr2", [B, NL * I], bf16, kind="ExternalInput").ap()
    eyef_d = nc.dram_tensor("eyef", [128, 128], f32, kind="ExternalInput").ap()
    eyeb_d = nc.dram_tensor("eyeb", [128, 128], bf16, kind="ExternalInput").ap()
    bdm_d = nc.dram_tensor("bdmask", [128, 128], bf16, kind="ExternalInput").ap()
    out_d = nc.dram_tensor("out", [B, C * D], f32, kind="ExternalOutput").ap()

    with tile.TileContext(nc) as tc:
        with (
            tc.tile_pool(name="const", bufs=1) as cp,
            tc.tile_pool(name="work", bufs=WP_BUFS) as wp,
            tc.tile_pool(name="rg", bufs=RP_BUFS) as rp,
            tc.tile_pool(name="dram", bufs=2, space="DRAM") as dp,
        ):
            sW2 = cp.tile([128, 4, CHUNKS, 512], bf16)
            sW3 = cp.tile([128, 4, NT, I, 8, D], bf16)
            sXT3 = cp.tile([128, I, NT, B], bf16)
            sXR2 = cp.tile([128, CHUNKS, 512], bf16)
            sEyeF = cp.tile([128, 128], f32)
            sEyeB = cp.tile([128, 128], bf16)
            sBdm = cp.tile([128, 128], bf16)

            # DMA-in: single queue in strict priority order — DMA transfers
            # serialize on the shared DMA-engine device, so arrival order is
            # consumption order: t0-cb0 inputs, then xr2/w2g0 (needed by the
            # first beta block), then alternating w3-cb / w2-g.
            w3v = sW3[:].rearrange("p cb a b c d -> p cb (a b c d)")
            qsz = NT * I * 8 * D
            w2v = sW2[:].rearrange("p g a b -> p g (a b)")
            gsz = CHUNKS * 512
            xrv = sXR2[:].rearrange("p a b -> p (a b)")
            nc.sync.dma_start(sXT3[:].rearrange("p a b c -> p (a b c)"), xt3_d[:])
            nc.sync.dma_start(sEyeB[:], eyeb_d[:])
            nc.sync.dma_start(sEyeF[:], eyef_d[:])
            nc.sync.dma_start(sBdm[:], bdm_d[:])
            ssz = qsz // 4
            for s_ in range(4):
                nc.sync.dma_start(w3v[:, 0, s_ * ssz:(s_ + 1) * ssz],
                                  w3_d[:, s_ * ssz:(s_ + 1) * ssz])
            nc.sync.dma_start(xrv[0:B, :], xr2_d[:])
            nc.sync.dma_start(xrv[B:128, :], xr2_d[:])
            nc.sync.dma_start(w2v[:, 0, :], w2_d[:, 0:gsz])
            for q_ in range(1, 4):
                nc.sync.dma_start(w3v[:, q_, :],
                                  w3_d[:, q_ * qsz:(q_ + 1) * qsz])
                nc.sync.dma_start(w2v[:, q_, :],
                                  w2_d[:, q_ * gsz:(q_ + 1) * gsz])

            sET = cp.tile([128, NT, C, B], bf16)
            sXt = cp.tile([128, I, NT, B], bf16)
            zA = cp.tile([128, NT, 16, B], bf16)
            zB = cp.tile([128, NT, 8, B], bf16)
            sZ = cp.tile([128, NT, B], bf16)
            sZr = cp.tile([128, NT, B], bf16)
            sS = cp.tile([B, C * D], f32)
            sSr = cp.tile([B, C * D], f32)
            sOut = cp.tile([B, C * D], f32)
            sOsum = cp.tile([B, C * D], f32)
            sOsumT = cp.tile([128, 4, B], bf16)
            sBDall = cp.tile([128, 16, 128], bf16)
            # squash temps (DVE-only; rsqrt via bit trick + Newton)
            sq = cp.tile([B, C * D], f32)
            s2 = cp.tile([B, C], f32)
            s2e = cp.tile([B, C], f32)
            ry = cp.tile([B, C], f32)
            rt = cp.tile([B, C], f32)
            opp = cp.tile([B, C], f32)
            rden = cp.tile([B, C], f32)
            fac = cp.tile([B, C], f32)

            nc.vector.memset(sBDall[:], 0.0)

            def squash_cb(src, dst, cb):
                sl = slice(cb * 128, (cb + 1) * 128)
                cs = slice(cb * 8, (cb + 1) * 8)
                nc.vector.tensor_mul(sq[:, sl], src[:, sl], src[:, sl])
                nc.vector.tensor_reduce(
                    s2[:, cs], sq[:, sl].rearrange("b (c j) -> b c j", j=D),
                    axis=AX.X, op=ALU.add)
                if SQUASH_EPS:
                    nc.vector.tensor_scalar_add(s2e[:, cs], s2[:, cs], EPS)
                    s2v = s2e
                else:
                    s2v = s2
                # rsqrt: quake seed + Newton (DVE-only, so ACT never
                # switches activation tables away from exp)
                yi = ry[:, cs].bitcast(i32)
                xi = s2v[:, cs].bitcast(i32)
                nc.vector.tensor_scalar(yi, xi, 1, None,
                                        op0=ALU.logical_shift_right)
                nc.vector.tensor_scalar(yi, yi, -1, MAGIC,
                                        op0=ALU.mult, op1=ALU.add)
                for _ in range(NEWTON_STEPS):
                    nc.vector.tensor_mul(rt[:, cs], ry[:, cs], ry[:, cs])
                    nc.vector.tensor_mul(rt[:, cs], rt[:, cs], s2v[:, cs])
                    nc.vector.tensor_scalar(rt[:, cs], rt[:, cs], -0.5, 1.5,
                                            op0=ALU.mult, op1=ALU.add)
                    nc.vector.tensor_mul(ry[:, cs], ry[:, cs], rt[:, cs])
                nc.vector.tensor_scalar_add(opp[:, cs], s2[:, cs], 1.0)
                nc.vector.reciprocal(rden[:, cs], opp[:, cs])
                nc.vector.tensor_mul(fac[:, cs], s2[:, cs], ry[:, cs])
                nc.vector.tensor_mul(fac[:, cs], fac[:, cs], rden[:, cs])
                fb = fac[:, cs].rearrange("b (c o) -> b c o", o=1) \
                    .broadcast_to([B, 8, D])
                nc.vector.tensor_mul(
                    dst[:, sl].rearrange("b (c j) -> b c j", j=D),
                    src[:, sl].rearrange("b (c j) -> b c j", j=D), fb)

            def all_reduce_cb(cb):
                sl = slice(cb * 128, (cb + 1) * 128)
                if sim:
                    nc.vector.tensor_copy(sSr[:, sl], sS[:, sl])
                else:
                    di = dp.tile([B, 128], f32, tag="ar_in")
                    do = dp.tile([B, 128], f32, tag="ar_out")
                    nc.sync.dma_start(di[:], sS[:, sl])
                    nc.gpsimd.collective_compute(
                        "AllReduce", mybir.AluOpType.add,
                        replica_groups=[list(range(NCORES))],
                        ins=[di[:].opt()], outs=[do[:].opt()])
                    nc.sync.dma_start(sSr[:, sl], do[:])

            # --- pipelined schedule -----------------------------------
            # per capsule-block cb: s-matmul(t, cb) -> drain -> AllReduce
            # -> squash -> osum -> BD(g=cb) -> NEXT iteration's beta units
            # for g=cb.  The only global barrier per iteration is softmax Z.

            with (
                tc.tile_pool(name="psA", bufs=2, space="PSUM") as psA,
                tc.tile_pool(name="psT", bufs=2, space="PSUM") as psT,
                tc.tile_pool(name="psS", bufs=1, space="PSUM") as psS,
            ):
                pending = [None]

                def emit_reduce():
                    if pending[0] is None:
                        return
                    g, p, pT2, tmp = pending[0]
                    pending[0] = None
                    c0 = g * 8 + 2 * p
                    with (tc.high_priority() if HP_EXP
                          else _nullctx()):
                        if EXP_SPLIT:
                            for h_ in range(2):
                                ns = slice(2 * h_, 2 * h_ + 2)
                                nc.scalar.activation(
                                    sET[:, ns, c0:c0 + 2, :]
                                       .rearrange("p nt a b -> p nt (a b)"),
                                    pT2[:, ns, :]
                                       .rearrange("p nt f -> p (nt f)"),
                                    AF.Exp)
                        else:
                            nc.scalar.activation(
                                sET[:, :, c0:c0 + 2, :]
                                   .rearrange("p nt a b -> p nt (a b)"),
                                pT2[:].rearrange("p nt f -> p (nt f)"), AF.Exp)
                        if g >= 2:
                            # level-1 Z: e[c] + e[c+16] for this p's pair
                            cl = (g - 2) * 8 + 2 * p
                            zeng = nc.gpsimd if L1_POOL else nc.vector
                            zeng.tensor_add(
                                zA[:, :, cl:cl + 2, :],
                                sET[:, :, cl:cl + 2, :],
                                sET[:, :, cl + 16:cl + 18, :])

                def emit_traccum_q(q):
                    g, p, pT2, tmp = pending[0]
                    t8 = tmp[:, 2 * q:2 * q + 2, :] \
                        .rearrange("p a b -> p (a b)") \
                        .rearrange("p (n i) -> p n i", i=I)
                    for i in range(I):
                        nc.tensor.matmul(
                            pT2[:, q, :], t8[:, :, i], sEyeB[:],
                            start=(i == 0), stop=(i == I - 1))

                def emit_pb_unit(g, p, qpath):
                    # one beta unit (capsule pair) of group g
                    u = g * 4 + p
                    pT2 = psT.tile([128, NT, 128], f32, name="pT2",
                                   tag="bT")
                    tmp = wp.tile([128, CHUNKS, 512], bf16, name="tmp",
                                  tag="tmp")
                    for q in range(4):
                        pA = psA.tile([128, 1024], f32, name="pA",
                                      tag="pA")
                        for h in range(2):
                            nc.tensor.matmul(
                                pA[:, 512 * h:512 * (h + 1)],
                                sBDall[:, u, :],
                                sW2[:, g, 2 * q + h, :],
                                start=True, stop=True)
                        path = qpath[(u * 4 + q) % 16]
                        tv = tmp[:, 2 * q:2 * q + 2, :] \
                            .rearrange("p a b -> p (a b)")
                        xv = sXR2[:, 2 * q:2 * q + 2, :] \
                            .rearrange("p a b -> p (a b)")
                        if path == "D":
                            nc.vector.tensor_mul(tv, pA[:], xv)
                        else:
                            nc.scalar.copy(tv, pA[:])
                            meng = nc.gpsimd if path == "P" else nc.vector
                            meng.tensor_mul(tv, tv, xv)
                        # previous unit's quarter-q i-reduce fills PE
                        # while this unit's x-mul is still running
                        if pending[0] is not None:
                            emit_traccum_q(q)
                            if q == 3:
                                emit_reduce()
                    pending[0] = (g, p, pT2, tmp)

                def flush_z():
                    if pending[0] is not None:
                        for q in range(4):
                            emit_traccum_q(q)
                        emit_reduce()
                    hpz = tc.high_priority() if HP_FLUSHZ else _nullctx()
                    hpz.__enter__()
                    nc.vector.tensor_add(zB[:], zA[:, :, 0:8, :],
                                         zA[:, :, 8:16, :])
                    nc.vector.tensor_add(zA[:, :, 0:4, :], zB[:, :, 0:4, :],
                                         zB[:, :, 4:8, :])
                    nc.vector.tensor_add(zB[:, :, 0:2, :], zA[:, :, 0:2, :],
                                         zA[:, :, 2:4, :])
                    nc.vector.tensor_add(
                        sZ[:].rearrange("p nt (o b) -> p nt o b", o=1),
                        zB[:, :, 0:1, :], zB[:, :, 1:2, :])
                    with nc.allow_low_precision(reason="Z~32 in bf16"):
                        nc.vector.reciprocal(sZr[:], sZ[:])
                    for h in range(2):
                        nt0, nt1 = h * 2, h * 2 + 2
                        zb = sZr[:, nt0:nt1, :] \
                            .rearrange("p (o nt) b -> p o nt b", o=1) \
                            .broadcast_to([128, I, 2, B])
                        nc.vector.tensor_mul(sXt[:, :, nt0:nt1, :],
                                             sXT3[:, :, nt0:nt1, :], zb)
                    hpz.__exit__(None, None, None)

                slot_rgs = {}

                def emit_rg(t, cb, irange):
                    # coupling * x products feeding the cb s-matmul
                    rgs = slot_rgs.setdefault(cb, [None] * I)
                    done = set()
                    for i in irange:
                        if i in done:
                            continue
                        pool = (cb, i) in RG_POOL_T[t]
                        j = i + 1
                        pairable = (RG_PAIR and not pool and j in irange
                                    and (cb, j) not in RG_POOL_T[t])
                        if pairable:
                            rg = rp.tile([128, 2, NT, 8, B], bf16,
                                         name=f"rgp{i}", tag=f"rg{i}")
                            x2 = sXt[:, i:i + 2, :, :] \
                                .rearrange("p i nt (o b) -> p i nt o b", o=1) \
                                .broadcast_to([128, 2, NT, 8, B])
                            e2 = sET[:, :, cb * 8:(cb + 1) * 8, :] \
                                .rearrange("p (o nt) c b -> p o nt c b", o=1) \
                                .broadcast_to([128, 2, NT, 8, B])
                            nc.vector.tensor_mul(rg[:], e2, x2)
                            rgs[i] = rg[:, 0]
                            rgs[j] = rg[:, 1]
                            done.add(j)
                            continue
                        rg = rp.tile([128, NT, 8, B], bf16,
                                     name=f"rg{i}", tag=f"rg{i}")
                        xb = sXt[:, i, :, :] \
                            .rearrange("p nt (o b) -> p nt o b", o=1) \
                            .broadcast_to([128, NT, 8, B])
                        if pool and POOL_RG_SPLIT:
                            for h_ in range(2):
                                ns = slice(2 * h_, 2 * h_ + 2)
                                nc.gpsimd.tensor_mul(
                                    rg[:, ns, :, :],
                                    sET[:, ns, cb * 8:(cb + 1) * 8, :],
                                    xb[:, ns, :, :])
                        else:
                            meng = nc.gpsimd if pool else nc.vector
                            meng.tensor_mul(
                                rg[:], sET[:, :, cb * 8:(cb + 1) * 8, :], xb)
                        rgs[i] = rg

                def emit_smm(t, cb):
                    # s-matmul for capsule block cb at routing step t
                    pS = psS.tile([B, 128], f32, name="pS", tag="sS")
                    if t == 0:
                        # nt-outer so cb0's first w3 sub-chunk DMA unblocks
                        # the first steps
                        step = 0
                        for nt in range(NT):
                            for i in range(I):
                                rhs = sW3[:, cb, nt, i, :, :] \
                                    .rearrange("p a b -> p (a b)")
                                nc.tensor.matmul(
                                    pS[:], sXT3[:, i, nt, :], rhs,
                                    start=(step == 0), stop=(step == 31))
                                step += 1
                        nc.scalar.mul(sS[:, cb * 128:(cb + 1) * 128], pS[:],
                                      1.0 / C)
                        return
                    rgs = slot_rgs.pop(cb)
                    # one accumulation group open per PSUM tile at a time
                    for c8 in range(8):
                        for i in range(I):
                            for nt in range(NT):
                                nc.tensor.matmul(
                                    pS[:, c8 * D:(c8 + 1) * D],
                                    rgs[i][:, nt, c8, :],
                                    sW3[:, cb, nt, i, c8, :],
                                    start=(i == 0 and nt == 0),
                                    stop=(i == I - 1 and nt == NT - 1))
                    nc.scalar.copy(sS[:, cb * 128:(cb + 1) * 128], pS[:])

                def emit_pd(t, cb):
                    if t > 0:
                        emit_rg(t, cb, range(I))
                    emit_smm(t, cb)

                def emit_tail(t, cb):
                    last = (t == ROUTINGS - 1)
                    hpt = tc.high_priority() if HP_TAIL else _nullctx()
                    hpt.__enter__()
                    all_reduce_cb(cb)
                    sl = slice(cb * 128, (cb + 1) * 128)
                    if t == 0:
                        squash_cb(sSr, sOsum, cb)
                    else:
                        squash_cb(sSr, sOut, cb)
                        if not last:
                            oeng = nc.gpsimd if OSUM_POOL else nc.vector
                            oeng.tensor_add(sOsum[:, sl], sOsum[:, sl],
                                            sOut[:, sl])
                    if last:
                        nc.sync.dma_start(out_d[:, sl], sOut[:, sl])
                        hpt.__exit__(None, None, None)
                        return
                    # transpose osum block, build BD tiles for group g=cb
                    pT = psT.tile([128, 4, B], f32, name="pT", tag="ot",
                                  bufs=1)
                    nc.tensor.transpose(pT[:, cb, :], sOsum[:, sl],
                                        sEyeF[0:B, 0:B])
                    nc.scalar.copy(sOsumT[:, cb, :], pT[:, cb, :])
                    g = cb
                    beng = nc.gpsimd if BD_POOL else nc.vector
                    for p in range(4):
                        ob = sOsumT[32 * p:32 * p + 32, g, :] \
                            .rearrange("p (o b) -> p o b", o=1) \
                            .broadcast_to([32, 2, B])
                        beng.tensor_mul(
                            sBDall[32 * p:32 * p + 32, g * 4 + p, :]
                                .rearrange("p (h b) -> p h b", h=2),
                            ob,
                            sBdm[32 * p:32 * p + 32, :]
                                .rearrange("p (h b) -> p h b", h=2))
                    hpt.__exit__(None, None, None)

                for t in range(ROUTINGS):
                    emit_pd(t, 0)
                    for cb in range(4):
                        emit_tail(t, cb)
                        if t < ROUTINGS - 1:
                            # next iteration's beta units for g=cb, with the
                            # next cb's Rg/s-matmul interleaved so no engine
                            # queue sees head-of-line blocking
                            nxt = cb + 1 if cb < 3 else None
                            for p in range(4):
                                emit_pb_unit(g=cb, p=p,
                                             qpath=QPATH_BY_T[t + 1])
                            if nxt is not None:
                                if t > 0:
                                    emit_rg(t, nxt, range(I))
                                emit_smm(t, nxt)
                        elif cb < 3:
                            emit_pd(t, cb + 1)
                    if t < ROUTINGS - 1:
                        flush_z()
    nc.compile()
    return nc


def get_nc(sim=False):
    key = "nc_sim" if sim else "nc"
    if key not in _CACHE:
        _CACHE[key] = _build_nc(sim=sim)
    return _CACHE[key]


def kernel(inputs, W):
    inputs = np.asarray(inputs, dtype=np.float32)
    W = np.asarray(W, dtype=np.float32)
    nc = get_nc()
    in_maps = host_prep_all(inputs, W)
    from concourse import bass_utils
    res = bass_utils.run_bass_kernel_spmd(
        nc, in_maps, core_ids=list(range(NCORES)))
    return res.results[0]["out"].reshape(B, C, D).astype(np.float32)
